# revision 54
# baseline (speedup 1.0000x reference)
"""CrissCross(actually full)-attention Trainium2 kernel.

Reference computation per batch b (C=64 channels, HW=4096 positions, D=8):
    q = Wq@x + bq        [D, HW]
    k = Wk@x + bk        [D, HW]
    v = Wv@x + bv        [C, HW]
    att[i, j] = softmax_i(q[:, i] . k[:, j])
    out[c, j] = sum_i v[c, i] att[i, j] + x[c, j]

Sharding: data-parallel, one batch per NeuronCore (8 cores).

Measured HW model (from NTFF traces of this kernel's runs):
  - QK group (3 row-tiled [8,128]x[8,512] f32r matmuls) is STREAM-bound:
    wall ~727 ns cold (K=4/8, 1.2 GHz) / ~435 ns warm (K=8/8, 2.4 GHz).
  - AV matmul ([128,65]bf16 stationary, 512-col stream): cadence 427 cold /
    241 warm.
  - ScalarE exp of [128, 1536]: 1573 ns cold-phase, 1423 measured in steady
    warm state (ScalarE ~1.33 GHz effective); 88 of them set the ~125-138 us
    ScalarE floor.  Per group: PE cold 2008 ns (PE-paced); PE warm 1158 <
    exp (ScalarE-paced).  Exps are irreducible: 16.7M per core on the only
    exp-capable engine.
  - PE_HAM: the un-throttle grant (K=4/8 -> 8/8) arrives 60-140 us into the
    run at a firmware-paced, effectively random time; early dummy-matmul
    warmup bursts do NOT move it (measured), and grant "blips" die if a PE
    stall lands inside the 3.4 us grant window -- hence the all-out war on
    pipeline gaps below.  Under sustained board heat a separate P0 state
    downclocks PE 2.4->2.0 and ScalarE ~1.33->1.1, inflating everything
    ~15-20%; run-to-run comparisons must account for it.

Per-core dataflow: x'=[x;ones] (biases folded via the ones row); q and k are
projected chunk-by-chunk with REPLICATED stationaries (weight columns at
{0-7,32-39,64-71}) so each 512-wide chunk lands with its row-tiling replicas
in place and one [72,512] DVE copy evacuates it (never-read garbage rows
ride along).  k chunk c is first read in j-tile c, so pk projections trickle
one per group through j-tile 0 and evacuate on ScalarE's slack; vT' (bf16,
trailing ones column -> AV emits numerator and denominator together) is
projected 4 blocks per tile with one-group lookahead.  Softmax skips
max-subtraction (|logit| < ~26, well inside fp32/bf16 exp range).

PSUM layout (single pool, 8 banks, explicit tags):
  qkA [128,1536] banks 0-2   |  exp-input ring, alternating per group
  qkB [128,1536] banks 3-5   |  (global group parity across j-tiles)
  avA [128, 512] bank 6      |  AV accumulator, alternating per j-tile
  avB [128, 512] bank 7      |  parity; j-tile 0's projection scratch
                                also rotates through these banks
Epilogue per j-tile (runs while the next j-tile computes, PE-free):
  DVE reciprocal reads the denominator row (av[64]) straight from PSUM; a
  partition-broadcast DMA (zero-step FREE dim; ~6.5 us descriptor latency,
  fully hidden) replicates it to 64 rows; DVE mult (+x residual) and DMA
  out.  The next j-tile accumulates into the other av bank, so the PE never
  stalls on the epilogue -- stall-free j-tile boundaries are what let HAM
  warm windows survive once granted.  The LAST j-tile instead broadcasts
  via a ones-stationary matmul into the now-idle qk ring (tile_position
  (64,0)) in two 256-column halves, hiding half the 3.3 us reciprocal.
"""

import numpy as np

import bass_rust
import concourse.bass as bass
import concourse.tile as tile
from concourse import mybir
from concourse.bass_utils import run_bass_kernel_spmd

B, C, HW, D = 8, 64, 4096, 8
H = W = 64
JT = 512          # j-tile width (PSUM bank)
NJ = HW // JT     # 8
IB = 128          # i-block height (partitions)
NI = HW // IB     # 32
GRP = 3           # i-blocks per exp group (3-way row tiling)
N_WARM = 0        # boot HAM-warmup dummy matmuls: measured useless (the
                  # PE_HAM un-throttle grant is firmware-paced, ~85-127 us
                  # into the run regardless of early PE activity)
VB = 4            # vT i-blocks projected per PSUM tile
AV_LAG = 2        # groups the AV flush trails the QK/exp front
TAIL_LAG = 2      # groups the PE bcast trails the epilogue's reciprocal

F32 = mybir.dt.float32
F32R = mybir.dt.float32r
F16 = mybir.dt.float16
BF16 = mybir.dt.bfloat16


def _fix_drain_waits(nc):
    """walrus in this container rejects instructions carrying more than one
    sync-wait; hoist extras onto NoOps inserted just before, same engine."""
    for f in nc.m.functions:
        for blk in f.blocks:
            insts = blk.instructions
            for tgt in [
                i for i in list(insts)
                if i.sync_info and len(i.sync_info.on_wait or []) > 1
            ]:
                si = tgt.sync_info
                waits = list(si.on_wait)
                si.on_wait = waits[-1:]
                di = insts.index(tgt)
                for w in waits[:-1]:
                    n = nc.engines[tgt.engine].nop()
                    for b in f.blocks:
                        bi = b.instructions
                        for idx in range(len(bi) - 1, -1, -1):
                            if bi[idx].name == n.ins.name:
                                bi.pop(idx)
                                break
                    n.ins.sync_info = bass_rust.SyncInfo(on_wait=[w], on_update=[])
                    insts.insert(di, n.ins)
                    di += 1


def build_nc(loop_n=None, bodies=1):
    nc = bass.Bass()
    x_d = nc.dram_tensor("x", [C, HW], F32, kind="ExternalInput")
    wq_d = nc.dram_tensor("Wq", [D, C], F32, kind="ExternalInput")
    bq_d = nc.dram_tensor("bq", [D], F32, kind="ExternalInput")
    wk_d = nc.dram_tensor("Wk", [D, C], F32, kind="ExternalInput")
    bk_d = nc.dram_tensor("bk", [D], F32, kind="ExternalInput")
    wv_d = nc.dram_tensor("Wv", [C, C], F32, kind="ExternalInput")
    bv_d = nc.dram_tensor("bv", [C], F32, kind="ExternalInput")
    out_d = nc.dram_tensor("out", [C, HW], F32, kind="ExternalOutput")

    with tile.TileContext(nc) as tc:
        with (
            tc.tile_pool(name="const", bufs=1) as cp,
            tc.tile_pool(name="work", bufs=4) as wp,
            tc.tile_pool(name="qtmp", bufs=2) as qp,
            tc.tile_pool(name="ps", bufs=1, space="PSUM") as pp,
        ):
            # ---- persistent SBUF tensors ----
            x_raw = cp.tile([C, HW], F32, tag="xraw")        # residual source
            x_sb = cp.tile([C + 1, HW], F32R, tag="x")       # x' = [x; ones]
            # raw weights land CONTIGUOUS and are transposed on-chip (DVE
            # 32x32 block transposes); transposing DMAs cost us of descriptors
            wqwk_raw = cp.tile([32, 2 * C], F32, tag="wqwkraw")
            wv_raw = cp.tile([C, C], F32, tag="wvraw")
            bias_raw = cp.tile([1, 2 * D + C], F32, tag="braw")
            wqkT = cp.tile([C, C], F32, tag="wqkT")            # [WqT | WkT]
            wvT = cp.tile([C, C], F32, tag="wvT")
            # replicated projection stationaries: weight columns at
            # {0-7, 32-39, 64-71} so the projection matmul emits q (resp. k)
            # with its row-tiling replicas already in place -- one [72,512]
            # DVE copy evacuates a whole chunk (cols 8-31/40-63 are never
            # read downstream, so they stay uninitialized)
            wq_rep = cp.tile([C + 1, GRP, 32], F32R, tag="wqrep")
            wk_rep = cp.tile([C + 1, GRP, 32], F32R, tag="wkrep")
            wv_sb = cp.tile([C + 1, C], F32R, tag="wv")        # [WvT; bv]
            q_sb = cp.tile([64 + D, HW], F32R, tag="q")    # replicas @0/32/64
            k_sb = cp.tile([64 + D, HW], F32R, tag="k")
            vt_sb = cp.tile([IB, NI, C + 1], BF16, tag="vt")   # vT' blocks
            ones_sb = cp.tile([IB, 1], F32, tag="ones")
            # tail-epilogue bcast stationary (row 64) + HAM-warmup operands
            ones_rows = cp.tile([IB, C], F32R, tag="onesrow")
            warm_rhs = cp.tile([1, JT], F32R, tag="warmrhs")
            warm_sb = cp.tile([1, 4], F32, tag="warm")         # act-table warmup

            # ---- boot: critical DMAs first, then DVE chain in dep order ----
            # x chunk 0 + q/k weights gate the first projection; they go at
            # the head of their queues.  Engine queues are in-order, so the
            # emission order below IS the issue order.
            nc.vector.memset(wqwk_raw[:, :], 0.0)
            nc.sync.dma_start(out=x_raw[:, 0:JT], in_=x_d[:, 0:JT])
            nc.sync.dma_start(out=wqwk_raw[0:D, 0:C], in_=wq_d[:, :])
            nc.sync.dma_start(out=wqwk_raw[0:D, C:2 * C], in_=wk_d[:, :])
            nc.sync.dma_start(out=bias_raw[0:1, 0:D], in_=bq_d[None, :])
            nc.sync.dma_start(out=bias_raw[0:1, D:2 * D], in_=bk_d[None, :])
            nc.scalar.dma_start(out=x_raw[:, JT:2 * JT], in_=x_d[:, JT:2 * JT])
            nc.scalar.dma_start(out=wv_raw[:, :], in_=wv_d[:, :])
            nc.scalar.dma_start(out=bias_raw[0:1, 2 * D:], in_=bv_d[None, :])
            # all x chunks ride HWDGE queues (sync/scalar): SWDGE's multi-us
            # spin-up latency on the gpsimd queue was gating the j-tile-0
            # x-round CASTs, stalling the projection chain
            nc.sync.dma_start(out=x_raw[:, 2 * JT:3 * JT],
                              in_=x_d[:, 2 * JT:3 * JT])
            nc.scalar.dma_start(out=x_raw[:, 3 * JT:4 * JT],
                                in_=x_d[:, 3 * JT:4 * JT])
            nc.sync.dma_start(out=x_raw[:, 4 * JT:5 * JT],
                              in_=x_d[:, 4 * JT:5 * JT])
            nc.scalar.dma_start(out=x_raw[:, 5 * JT:6 * JT],
                                in_=x_d[:, 5 * JT:6 * JT])
            nc.sync.dma_start(out=x_raw[:, 6 * JT:7 * JT],
                              in_=x_d[:, 6 * JT:7 * JT])
            nc.scalar.dma_start(out=x_raw[:, 7 * JT:8 * JT],
                                in_=x_d[:, 7 * JT:8 * JT])
            # constants on GpSimd (keeps the DVE queue free for the boot
            # critical chain); wqk_sb zero covers the unused cols 8-31
            nc.gpsimd.memset(ones_sb[:, :], 1.0)
            nc.gpsimd.memset(vt_sb[:, :, C:C + 1], 1.0)
            nc.gpsimd.memset(ones_rows[:, :].bitcast(F32), 1.0)
            nc.gpsimd.memset(wq_rep[:, :, :].bitcast(F32), 0.0)
            nc.gpsimd.memset(wk_rep[:, :, :].bitcast(F32), 0.0)
            # pre-load the ScalarE activation table during the DMA wait
            nc.scalar.activation(warm_sb[0:1, :],
                                 ones_sb[0:1, 0:1].to_broadcast([1, 4]),
                                 mybir.ActivationFunctionType.Exp)
            # HAM warmup: PE_HAM un-throttles only after a long stretch of
            # busy activity windows; burn the otherwise-idle boot DMA wait on
            # dummy matmuls so the busy counter starts at ~1 us, not ~15 us
            if N_WARM:
                warm_ps = pp.tile([IB, GRP * JT], F32, tag="qkA",
                                  name="warmps")
                for i in range(N_WARM):
                    nc.tensor.matmul(warm_ps[0:C, 0:JT],
                                     lhsT=ones_rows[0:1, :],
                                     rhs=warm_rhs[0:1, :],
                                     start=(i == 0), stop=(i == N_WARM - 1))
            # on-chip transposes: Wq/Wk rows live in wqwk_raw[0:8] (rest
            # zeroed); DVE transposes 32x32 blocks
            for m in range(2):          # 0 = q, 1 = k
                for j in range(2):
                    nc.vector.transpose(
                        wqkT[32 * j:32 * j + 32, 32 * m:32 * m + 32],
                        wqwk_raw[0:32, m * C + 32 * j:m * C + 32 * j + 32])
            # free-dim-broadcast copies place each weight block at the three
            # 32-aligned column positions in one DVE instruction
            nc.vector.tensor_copy(
                wq_rep[0:C, :, 0:D],
                wqkT[0:C, None, 0:D].to_broadcast((C, GRP, D)))
            nc.vector.tensor_copy(
                wq_rep[C:C + 1, :, 0:D],
                bias_raw[0:1, None, 0:D].to_broadcast((1, GRP, D)))
            nc.vector.tensor_copy(
                wk_rep[0:C, :, 0:D],
                wqkT[0:C, None, 32:32 + D].to_broadcast((C, GRP, D)))
            nc.vector.tensor_copy(
                wk_rep[C:C + 1, :, 0:D],
                bias_raw[0:1, None, D:2 * D].to_broadcast((1, GRP, D)))

            x_rounded = [False] * 8
            wv_init = [False]

            def ensure_x(ch):
                """Round x chunk ch (512 wide) to f32r lazily so the startup
                chain doesn't queue behind the whole x preprocessing."""
                cs = slice(ch * JT, (ch + 1) * JT)
                if x_rounded[ch]:
                    return
                x_rounded[ch] = True
                nc.vector.tensor_copy(x_sb[0:C, cs], x_raw[:, cs])
                nc.gpsimd.memset(x_sb[C:C + 1, cs].bitcast(F32), 1.0)

            def emit_proj(ct, w_rep, dst, proj_tile, on_scalar=False):
                """Project one tensor (q or k) for HW-chunk ct (512 wide);
                the replicated stationary lands it at partition groups
                {0, 32, 64} directly, so one [72,512] DVE copy evacuates the
                chunk (garbage rows 8-31/40-63 go along for the ride into
                never-read q_sb/k_sb rows)."""
                ensure_x(ct)
                js = slice(ct * JT, (ct + 1) * JT)
                p = proj_tile()
                nc.tensor.matmul(p[0:64 + D, :],
                                 lhsT=w_rep[:, :, :].rearrange(
                                     "p g c -> p (g c)")[:, 0:64 + D],
                                 rhs=x_sb[:, js], start=True, stop=True)
                if on_scalar:
                    # k chunks c>=1 are first read in j-tile c; their
                    # evacuation rides ScalarE's jt0 slack instead of the
                    # DVE queue (which gates the q/vt critical chains)
                    nc.scalar.activation(dst[:, js], p[0:64 + D, :],
                                         mybir.ActivationFunctionType.Copy)
                else:
                    nc.vector.tensor_copy(dst[:, js], p[0:64 + D, :])

            def emit_vt_proj(vb, proj_tile):
                """Project vT' i-blocks vb*VB .. vb*VB+VB-1."""
                if not wv_init[0]:
                    wv_init[0] = True
                    for i in range(2):
                        for j in range(2):
                            nc.vector.transpose(
                                wvT[32 * j:32 * j + 32, 32 * i:32 * i + 32],
                                wv_raw[32 * i:32 * i + 32, 32 * j:32 * j + 32])
                    nc.vector.tensor_copy(wv_sb[0:C, :], wvT[:, :])
                    nc.vector.tensor_copy(wv_sb[C:C + 1, :],
                                          bias_raw[0:1, 2 * D:])
                ensure_x((vb * VB * IB) // JT)
                ensure_x(((vb + 1) * VB * IB - 1) // JT)
                pv = proj_tile()
                for u in range(VB):
                    ib = vb * VB + u
                    isl = slice(ib * IB, (ib + 1) * IB)
                    nc.tensor.matmul(pv[0:IB, u * C:(u + 1) * C],
                                     lhsT=x_sb[:, isl], rhs=wv_sb[:, :],
                                     start=True, stop=True)
                nc.vector.tensor_copy(
                    vt_sb[:, vb * VB:(vb + 1) * VB, 0:C],
                    pv[:, 0:VB * C].rearrange("p (v c) -> p v c", v=VB))

            def _compute():
                n_grp = (NI + GRP - 1) // GRP
                qk_done = 0
                vt_done = 0
                gidx = [0]         # global group counter (qk ring parity)
                step = [0]
                pend_av = []       # FIFO of (av, att, g, nb, js)
                pend_ep = []       # (av, js)

                def qk_tile():
                    # fp32 logits: TRN2 matmul can only write fp32 PSUM
                    # (16-bit PSUM output is TRN3-only), which pins the exp
                    # ring at 2x3 banks and GRP at 3
                    t = pp.tile([IB, GRP * JT], F32,
                                tag=("qkA" if gidx[0] % 2 == 0 else "qkB"),
                                name="qk")
                    gidx[0] += 1
                    return t

                def av_tile(jt):
                    return pp.tile([IB, JT], F32,
                                   tag=("avA" if jt % 2 == 0 else "avB"),
                                   name="av")

                def proj_tile():
                    # j-tile 0 projection scratch shares avB (av(jt1) is the
                    # next user of that bank, long after the last projection)
                    return pp.tile([IB, JT], F32, tag="avB", name="proj")

                def proj_tileA():
                    # chunk 0's k projection rides the avA bank, which is
                    # free until av(jt0)'s first accumulation at step 2 --
                    # this keeps it off pq(0)'s WAR chain so the first QK
                    # group isn't serialized behind two evacuations
                    return pp.tile([IB, JT], F32, tag="avA", name="projA")

                def ensure_vt(hi_block):
                    nonlocal vt_done
                    while vt_done * VB < hi_block:
                        emit_vt_proj(vt_done, proj_tile)
                        vt_done += 1

                def flush_av():
                    pav, patt, pg, pnb, pjs = pend_av.pop(0)
                    # one-group vt lookahead so the DVE evacuation is queued
                    # well before the AV matmuls that read it
                    ensure_vt(min(NI, pg * GRP + pnb + GRP))
                    for bi in range(pnb):
                        ib = pg * GRP + bi
                        nc.tensor.matmul(
                            pav[0:C + 1, :],
                            lhsT=vt_sb[:, ib, :],
                            rhs=patt[:, bi * JT:(bi + 1) * JT],
                            start=(ib == 0), stop=(ib == NI - 1))
                    if pg * GRP + pnb == NI:
                        pend_ep.append((pav, pjs))

                def flush_ep(final=False):
                    while pend_ep:
                        pav, pjs = pend_ep.pop(0)
                        # reciprocal straight off the PSUM denominator row
                        # (no den evacuation copy); the next j-tile
                        # accumulates into the other av bank, so the PE never
                        # stalls on this epilogue
                        if final:
                            # tail path: the qk ring is idle now, so a ones-
                            # stationary matmul broadcasts the reciprocal into
                            # a ring bank in ~0.5 us (the DMA broadcast below
                            # costs ~6.5 us of descriptor latency, hidden
                            # mid-run but fully exposed at the tail).  The
                            # 3.3 us DVE reciprocal is the tail's long pole,
                            # so run the epilogue in two 256-column halves:
                            # half 1's bcast/mult/add/DMA overlap half 2's
                            # reciprocal.
                            recip_r = wp.tile([IB, JT], F32R, tag="recipr")
                            bcps = pp.tile([IB, JT], F32, tag="qkA",
                                           name="bcps")
                            HJ = JT // 2
                            for h in range(2):
                                hs = slice(h * HJ, (h + 1) * HJ)
                                with nc.allow_low_precision(
                                        reason="f32r round of softmax recip"):
                                    nc.vector.reciprocal(recip_r[64:65, hs],
                                                         pav[64:65, hs])
                                nc.tensor.matmul(bcps[0:C, hs],
                                                 lhsT=ones_rows[64:65, 0:C],
                                                 rhs=recip_r[64:65, hs],
                                                 start=True, stop=True,
                                                 tile_position=(64, 0))
                            for h in range(2):
                                hs = slice(h * HJ, (h + 1) * HJ)
                                hjs = slice(pjs.start + h * HJ,
                                            pjs.start + (h + 1) * HJ)
                                bc_sb = wp.tile([C, HJ], F32, tag="bch")
                                nc.vector.tensor_copy(bc_sb[:, :],
                                                      bcps[0:C, hs])
                                oh = wp.tile([C, HJ], F32, tag="oh")
                                nc.vector.tensor_tensor(
                                    oh[:, :], pav[0:C, hs], bc_sb[:, :],
                                    op=mybir.AluOpType.mult)
                                nc.vector.tensor_tensor(
                                    oh[:, :], oh[:, :], x_raw[:, hjs],
                                    op=mybir.AluOpType.add)
                                nc.sync.dma_start(out=out_d[:, hjs],
                                                  in_=oh[:, :])
                            continue
                        else:
                            recip = wp.tile([IB, JT], F32, tag="recip")
                            nc.vector.reciprocal(recip[64:65, :],
                                                 pav[64:65, :])
                            bc_sb = wp.tile([C, JT], F32, tag="bc")
                            # partition-broadcast DMA: zero-step FREE dim on
                            # the src (the same 2 KB row read 64 times); a
                            # zero-step PARTITION dim is rejected by the DMA
                            # lowering.  ~6.5 us of completion latency, fully
                            # hidden by the next j-tile's compute
                            nc.sync.dma_start(
                                out=bc_sb[:, :],
                                in_=recip[64:65, None, :]
                                .to_broadcast((1, C, JT)))
                            bc_src = bc_sb[:, :]
                        o = wp.tile([C, JT], F32, tag="o")
                        nc.vector.tensor_tensor(o[:, :], pav[0:C, :],
                                                bc_src,
                                                op=mybir.AluOpType.mult)
                        nc.vector.tensor_tensor(o[:, :], o[:, :], x_raw[:, pjs],
                                                op=mybir.AluOpType.add)
                        nc.sync.dma_start(out=out_d[:, pjs], in_=o[:, :])

                # prologue: chunk-0 projections precede av(jt0)'s allocation
                # in both bank rings
                emit_proj(0, wq_rep, q_sb, proj_tile)
                emit_proj(0, wk_rep, k_sb, proj_tileA)
                qk_done = 1
                k_done = 1

                for jt in range(NJ):
                    js = slice(jt * JT, (jt + 1) * JT)
                    av = av_tile(jt)
                    for g in range(n_grp):
                        nb = min(GRP, NI - g * GRP)
                        if jt == 0:
                            # just-in-time q projections, one chunk ahead of
                            # the QK front; k chunk c is only read from
                            # j-tile c on, so the pk projections trickle one
                            # per group (halving early proj-bank WAR stalls)
                            hi_i = (g * GRP + nb) * IB
                            need = min(8, max(1, -(-hi_i // JT)) + 1)
                            while qk_done < need:
                                emit_proj(qk_done, wq_rep, q_sb, proj_tile)
                                qk_done += 1
                            if k_done < 2 and g >= 1:
                                # only k chunk 1 is needed before j-tile 1;
                                # chunks 2-7 are projected later, one per
                                # j-tile, in the idle opposite-parity av bank
                                emit_proj(k_done, wk_rep, k_sb, proj_tile)
                                k_done += 1
                            if g == n_grp - 1:
                                while qk_done < 8:
                                    emit_proj(qk_done, wq_rep, q_sb,
                                              proj_tile)
                                    qk_done += 1
                                pass
                                # guarantee every avB-bank projection tile is
                                # emitted before av(jt1)'s allocation (the
                                # lazy flush_av path already reaches 32 here)
                                ensure_vt(NI)
                        qk = qk_tile()
                        for bi in range(nb):
                            ib = g * GRP + bi
                            isl = slice(ib * IB, (ib + 1) * IB)
                            nc.tensor.matmul(
                                qk[:, bi * JT:(bi + 1) * JT],
                                lhsT=q_sb[32 * bi:32 * bi + D, isl],
                                rhs=k_sb[32 * bi:32 * bi + D, js],
                                start=True, stop=True,
                                tile_position=(32 * bi, 0))
                        if 1 <= jt <= 6 and g == 9 and k_done < 8:
                            emit_proj(k_done, wk_rep, k_sb,
                                      lambda: pp.tile(
                                          [IB, JT], F32,
                                          tag=("avA" if (jt + 1) % 2 == 0
                                               else "avB"),
                                          name="pklate"))
                            k_done += 1
                        att = wp.tile([IB, GRP * JT], BF16, tag="att")
                        nc.scalar.activation(
                            att[:, 0:nb * JT], qk[:, 0:nb * JT],
                            mybir.ActivationFunctionType.Exp)
                        flush_ep()
                        pend_av.append((av, att, g, nb, js))
                        while len(pend_av) > AV_LAG:
                            flush_av()
                        step[0] += 1
                while pend_av:
                    flush_av()
                    flush_ep(final=True)
                flush_ep(final=True)

            if loop_n:
                hints = (mybir.EngineType.PE, mybir.EngineType.Activation,
                         mybir.EngineType.DVE, mybir.EngineType.SP,
                         mybir.EngineType.Pool)
                with tc.For_i(0, loop_n, 1, hint_engines=hints):
                    for _ in range(bodies):
                        x_rounded[:] = [False] * 8
                        _compute()
            else:
                _compute()

    _fix_drain_waits(nc)
    return nc


_NC_CACHE = {}


def _get_nc():
    if "nc" not in _NC_CACHE:
        _NC_CACHE["nc"] = build_nc()
    return _NC_CACHE["nc"]


def kernel(**inputs) -> np.ndarray:
    x = np.ascontiguousarray(np.asarray(inputs["x"], dtype=np.float32))
    assert x.shape == (B, C, H, W), x.shape
    weights = {
        name: np.ascontiguousarray(np.asarray(inputs[name], dtype=np.float32))
        for name in ("Wq", "bq", "Wk", "bk", "Wv", "bv")
    }
    in_maps = [{"x": x[b].reshape(C, HW), **weights} for b in range(B)]
    nc = _get_nc()
    res = run_bass_kernel_spmd(nc, in_maps, core_ids=list(range(B)))
    out = np.stack([np.asarray(res.results[b]["out"]).reshape(C, H, W)
                    for b in range(B)])
    return out.astype(np.float32)


# revision 57
# speedup vs baseline: 1.0696x; 1.0696x over previous
"""CrissCross(actually full)-attention Trainium2 kernel.

Reference computation per batch b (C=64 channels, HW=4096 positions, D=8):
    q = Wq@x + bq        [D, HW]
    k = Wk@x + bk        [D, HW]
    v = Wv@x + bv        [C, HW]
    att[i, j] = softmax_i(q[:, i] . k[:, j])
    out[c, j] = sum_i v[c, i] att[i, j] + x[c, j]

Sharding: data-parallel, one batch per NeuronCore (8 cores).

Measured HW model (from NTFF traces of this kernel's runs):
  - QK group (3 row-tiled [8,128]x[8,512] f32r matmuls) is STREAM-bound:
    wall ~727 ns cold (K=4/8, 1.2 GHz) / ~435 ns warm (K=8/8, 2.4 GHz).
  - AV matmul ([128,65]bf16 stationary, 512-col stream): cadence 427 cold /
    241 warm.
  - ScalarE exp of [128, 1536]: 1573 ns cold-phase, 1423 measured in steady
    warm state (ScalarE ~1.33 GHz effective); 88 of them set the ~125-138 us
    ScalarE floor.  Per group: PE cold 2008 ns (PE-paced); PE warm 1158 <
    exp (ScalarE-paced).  Exps are irreducible: 16.7M per core on the only
    exp-capable engine.
  - PE_HAM: the un-throttle grant (K=4/8 -> 8/8) arrives 60-140 us into the
    run at a firmware-paced, effectively random time; early dummy-matmul
    warmup bursts do NOT move it (measured), and grant "blips" die if a PE
    stall lands inside the 3.4 us grant window -- hence the all-out war on
    pipeline gaps below.  Under sustained board heat a separate P0 state
    downclocks PE 2.4->2.0 and ScalarE ~1.33->1.1, inflating everything
    ~15-20%; run-to-run comparisons must account for it.

Per-core dataflow: x'=[x;ones] (biases folded via the ones row); q and k are
projected chunk-by-chunk with REPLICATED stationaries (weight columns at
{0-7,32-39,64-71}) so each 512-wide chunk lands with its row-tiling replicas
in place and one [72,512] DVE copy evacuates it (never-read garbage rows
ride along).  k chunk c is first read in j-tile c, so pk projections trickle
one per group through j-tile 0 and evacuate on ScalarE's slack; vT' (bf16,
trailing ones column -> AV emits numerator and denominator together) is
projected 4 blocks per tile with one-group lookahead.  Softmax skips
max-subtraction (|logit| < ~26, well inside fp32/bf16 exp range).

PSUM layout (single pool, 8 banks, explicit tags):
  qkA [128,1536] banks 0-2   |  exp-input ring, alternating per group
  qkB [128,1536] banks 3-5   |  (global group parity across j-tiles)
  avA [128, 512] bank 6      |  AV accumulator, alternating per j-tile
  avB [128, 512] bank 7      |  parity; j-tile 0's projection scratch
                                also rotates through these banks
Epilogue per j-tile (runs while the next j-tile computes, PE-free):
  DVE reciprocal reads the denominator row (av[64]) straight from PSUM; a
  partition-broadcast DMA (zero-step FREE dim; ~6.5 us descriptor latency,
  fully hidden) replicates it to 64 rows; DVE mult (+x residual) and DMA
  out.  The next j-tile accumulates into the other av bank, so the PE never
  stalls on the epilogue -- stall-free j-tile boundaries are what let HAM
  warm windows survive once granted.  The LAST j-tile instead broadcasts
  via a ones-stationary matmul into the now-idle qk ring (tile_position
  (64,0)) in two 256-column halves, hiding half the 3.3 us reciprocal.
"""

import numpy as np

import bass_rust
import concourse.bass as bass
import concourse.tile as tile
from concourse import mybir
from concourse.bass_utils import run_bass_kernel_spmd

B, C, HW, D = 8, 64, 4096, 8
H = W = 64
JT = 512          # j-tile width (PSUM bank)
NJ = HW // JT     # 8
IB = 128          # i-block height (partitions)
NI = HW // IB     # 32
GRP = 3           # i-blocks per exp group (3-way row tiling)
N_WARM = 0        # boot HAM-warmup dummy matmuls: measured useless (the
                  # PE_HAM un-throttle grant is firmware-paced, ~85-127 us
                  # into the run regardless of early PE activity)
VB = 4            # vT i-blocks projected per PSUM tile
AV_LAG = 2        # groups the AV flush trails the QK/exp front
TAIL_LAG = 2      # groups the PE bcast trails the epilogue's reciprocal

F32 = mybir.dt.float32
F32R = mybir.dt.float32r
F16 = mybir.dt.float16
BF16 = mybir.dt.bfloat16


def _fix_drain_waits(nc):
    """walrus in this container rejects instructions carrying more than one
    sync-wait; hoist extras onto NoOps inserted just before, same engine."""
    for f in nc.m.functions:
        for blk in f.blocks:
            insts = blk.instructions
            for tgt in [
                i for i in list(insts)
                if i.sync_info and len(i.sync_info.on_wait or []) > 1
            ]:
                si = tgt.sync_info
                waits = list(si.on_wait)
                si.on_wait = waits[-1:]
                di = insts.index(tgt)
                for w in waits[:-1]:
                    n = nc.engines[tgt.engine].nop()
                    for b in f.blocks:
                        bi = b.instructions
                        for idx in range(len(bi) - 1, -1, -1):
                            if bi[idx].name == n.ins.name:
                                bi.pop(idx)
                                break
                    n.ins.sync_info = bass_rust.SyncInfo(on_wait=[w], on_update=[])
                    insts.insert(di, n.ins)
                    di += 1


def build_nc(loop_n=None, bodies=1):
    nc = bass.Bass()
    x_d = nc.dram_tensor("x", [C, HW], F32, kind="ExternalInput")
    wq_d = nc.dram_tensor("Wq", [D, C], F32, kind="ExternalInput")
    bq_d = nc.dram_tensor("bq", [D], F32, kind="ExternalInput")
    wk_d = nc.dram_tensor("Wk", [D, C], F32, kind="ExternalInput")
    bk_d = nc.dram_tensor("bk", [D], F32, kind="ExternalInput")
    wv_d = nc.dram_tensor("Wv", [C, C], F32, kind="ExternalInput")
    bv_d = nc.dram_tensor("bv", [C], F32, kind="ExternalInput")
    out_d = nc.dram_tensor("out", [C, HW], F32, kind="ExternalOutput")

    with tile.TileContext(nc) as tc:
        with (
            tc.tile_pool(name="const", bufs=1) as cp,
            tc.tile_pool(name="work", bufs=4) as wp,
            tc.tile_pool(name="qtmp", bufs=2) as qp,
            tc.tile_pool(name="ps", bufs=1, space="PSUM") as pp,
        ):
            # ---- persistent SBUF tensors ----
            x_raw = cp.tile([C, HW], F32, tag="xraw")        # residual source
            x_sb = cp.tile([C + 1, HW], F32R, tag="x")       # x' = [x; ones]
            # raw weights land CONTIGUOUS and are transposed on-chip (DVE
            # 32x32 block transposes); transposing DMAs cost us of descriptors
            wqwk_raw = cp.tile([32, 2 * C], F32, tag="wqwkraw")
            wv_raw = cp.tile([C, C], F32, tag="wvraw")
            bias_raw = cp.tile([1, 2 * D + C], F32, tag="braw")
            wqkT = cp.tile([C, C], F32, tag="wqkT")            # [WqT | WkT]
            wvT = cp.tile([C, C], F32, tag="wvT")
            # replicated projection stationaries: weight columns at
            # {0-7, 32-39, 64-71} so the projection matmul emits q (resp. k)
            # with its row-tiling replicas already in place -- one [72,512]
            # DVE copy evacuates a whole chunk (cols 8-31/40-63 are never
            # read downstream, so they stay uninitialized)
            wq_rep = cp.tile([C + 1, GRP, 32], F32R, tag="wqrep")
            wk_rep = cp.tile([C + 1, GRP, 32], F32R, tag="wkrep")
            wv_sb = cp.tile([C + 1, C], F32R, tag="wv")        # [WvT; bv]
            q_sb = cp.tile([64 + D, HW], F32R, tag="q")    # replicas @0/32/64
            k_sb = cp.tile([64 + D, HW], F32R, tag="k")
            vt_sb = cp.tile([IB, NI, C + 1], BF16, tag="vt")   # vT' blocks
            ones_sb = cp.tile([IB, 1], F32, tag="ones")
            # tail-epilogue bcast stationary (row 64) + HAM-warmup operands
            ones_rows = cp.tile([IB, C], F32R, tag="onesrow")
            warm_rhs = cp.tile([1, JT], F32R, tag="warmrhs")
            warm_sb = cp.tile([1, 4], F32, tag="warm")         # act-table warmup

            # ---- boot: critical DMAs first, then DVE chain in dep order ----
            # x chunk 0 + q/k weights gate the first projection; they go at
            # the head of their queues.  Engine queues are in-order, so the
            # emission order below IS the issue order.
            nc.vector.memset(wqwk_raw[:, :], 0.0)
            nc.sync.dma_start(out=x_raw[:, 0:JT], in_=x_d[:, 0:JT])
            nc.sync.dma_start(out=wqwk_raw[0:D, 0:C], in_=wq_d[:, :])
            nc.sync.dma_start(out=wqwk_raw[0:D, C:2 * C], in_=wk_d[:, :])
            nc.sync.dma_start(out=bias_raw[0:1, 0:D], in_=bq_d[None, :])
            nc.sync.dma_start(out=bias_raw[0:1, D:2 * D], in_=bk_d[None, :])
            nc.scalar.dma_start(out=x_raw[:, JT:2 * JT], in_=x_d[:, JT:2 * JT])
            nc.scalar.dma_start(out=wv_raw[:, :], in_=wv_d[:, :])
            nc.scalar.dma_start(out=bias_raw[0:1, 2 * D:], in_=bv_d[None, :])
            # all x chunks ride HWDGE queues (sync/scalar): SWDGE's multi-us
            # spin-up latency on the gpsimd queue was gating the j-tile-0
            # x-round CASTs, stalling the projection chain
            nc.sync.dma_start(out=x_raw[:, 2 * JT:3 * JT],
                              in_=x_d[:, 2 * JT:3 * JT])
            nc.scalar.dma_start(out=x_raw[:, 3 * JT:4 * JT],
                                in_=x_d[:, 3 * JT:4 * JT])
            nc.sync.dma_start(out=x_raw[:, 4 * JT:5 * JT],
                              in_=x_d[:, 4 * JT:5 * JT])
            nc.scalar.dma_start(out=x_raw[:, 5 * JT:6 * JT],
                                in_=x_d[:, 5 * JT:6 * JT])
            nc.sync.dma_start(out=x_raw[:, 6 * JT:7 * JT],
                              in_=x_d[:, 6 * JT:7 * JT])
            nc.scalar.dma_start(out=x_raw[:, 7 * JT:8 * JT],
                                in_=x_d[:, 7 * JT:8 * JT])
            # constants on GpSimd (keeps the DVE queue free for the boot
            # critical chain); wqk_sb zero covers the unused cols 8-31
            nc.gpsimd.memset(ones_sb[:, :], 1.0)
            nc.gpsimd.memset(vt_sb[:, :, C:C + 1], 1.0)
            nc.gpsimd.memset(ones_rows[:, :].bitcast(F32), 1.0)
            nc.gpsimd.memset(wq_rep[:, :, :].bitcast(F32), 0.0)
            nc.gpsimd.memset(wk_rep[:, :, :].bitcast(F32), 0.0)
            # pre-load the ScalarE activation table during the DMA wait
            nc.scalar.activation(warm_sb[0:1, :],
                                 ones_sb[0:1, 0:1].to_broadcast([1, 4]),
                                 mybir.ActivationFunctionType.Exp)
            # HAM warmup: PE_HAM un-throttles only after a long stretch of
            # busy activity windows; burn the otherwise-idle boot DMA wait on
            # dummy matmuls so the busy counter starts at ~1 us, not ~15 us
            if N_WARM:
                warm_ps = pp.tile([IB, GRP * JT], F32, tag="qkA",
                                  name="warmps")
                for i in range(N_WARM):
                    nc.tensor.matmul(warm_ps[0:C, 0:JT],
                                     lhsT=ones_rows[0:1, :],
                                     rhs=warm_rhs[0:1, :],
                                     start=(i == 0), stop=(i == N_WARM - 1))
            # on-chip transposes: Wq/Wk rows live in wqwk_raw[0:8] (rest
            # zeroed); DVE transposes 32x32 blocks
            for m in range(2):          # 0 = q, 1 = k
                for j in range(2):
                    nc.vector.transpose(
                        wqkT[32 * j:32 * j + 32, 32 * m:32 * m + 32],
                        wqwk_raw[0:32, m * C + 32 * j:m * C + 32 * j + 32])
            # free-dim-broadcast copies place each weight block at the three
            # 32-aligned column positions in one DVE instruction
            nc.vector.tensor_copy(
                wq_rep[0:C, :, 0:D],
                wqkT[0:C, None, 0:D].to_broadcast((C, GRP, D)))
            nc.vector.tensor_copy(
                wq_rep[C:C + 1, :, 0:D],
                bias_raw[0:1, None, 0:D].to_broadcast((1, GRP, D)))
            nc.vector.tensor_copy(
                wk_rep[0:C, :, 0:D],
                wqkT[0:C, None, 32:32 + D].to_broadcast((C, GRP, D)))
            nc.vector.tensor_copy(
                wk_rep[C:C + 1, :, 0:D],
                bias_raw[0:1, None, D:2 * D].to_broadcast((1, GRP, D)))

            x_rounded = [False] * 8
            wv_init = [False]

            def ensure_x(ch):
                """Round x chunk ch (512 wide) to f32r lazily so the startup
                chain doesn't queue behind the whole x preprocessing."""
                cs = slice(ch * JT, (ch + 1) * JT)
                if x_rounded[ch]:
                    return
                x_rounded[ch] = True
                nc.vector.tensor_copy(x_sb[0:C, cs], x_raw[:, cs])
                nc.gpsimd.memset(x_sb[C:C + 1, cs].bitcast(F32), 1.0)

            def emit_proj(ct, w_rep, dst, proj_tile, on_scalar=False):
                """Project one tensor (q or k) for HW-chunk ct (512 wide);
                the replicated stationary lands it at partition groups
                {0, 32, 64} directly, so one [72,512] DVE copy evacuates the
                chunk (garbage rows 8-31/40-63 go along for the ride into
                never-read q_sb/k_sb rows)."""
                ensure_x(ct)
                js = slice(ct * JT, (ct + 1) * JT)
                p = proj_tile()
                nc.tensor.matmul(p[0:64 + D, :],
                                 lhsT=w_rep[:, :, :].rearrange(
                                     "p g c -> p (g c)")[:, 0:64 + D],
                                 rhs=x_sb[:, js], start=True, stop=True)
                if on_scalar:
                    # k chunks c>=1 are first read in j-tile c; their
                    # evacuation rides ScalarE's jt0 slack instead of the
                    # DVE queue (which gates the q/vt critical chains)
                    nc.scalar.activation(dst[:, js], p[0:64 + D, :],
                                         mybir.ActivationFunctionType.Copy)
                else:
                    nc.vector.tensor_copy(dst[:, js], p[0:64 + D, :])

            def emit_vt_proj(vb, proj_tile):
                """Project vT' i-blocks vb*VB .. vb*VB+VB-1."""
                if not wv_init[0]:
                    wv_init[0] = True
                    for i in range(2):
                        for j in range(2):
                            nc.vector.transpose(
                                wvT[32 * j:32 * j + 32, 32 * i:32 * i + 32],
                                wv_raw[32 * i:32 * i + 32, 32 * j:32 * j + 32])
                    nc.vector.tensor_copy(wv_sb[0:C, :], wvT[:, :])
                    nc.vector.tensor_copy(wv_sb[C:C + 1, :],
                                          bias_raw[0:1, 2 * D:])
                ensure_x((vb * VB * IB) // JT)
                ensure_x(((vb + 1) * VB * IB - 1) // JT)
                pv = proj_tile()
                for u in range(VB):
                    ib = vb * VB + u
                    isl = slice(ib * IB, (ib + 1) * IB)
                    nc.tensor.matmul(pv[0:IB, u * C:(u + 1) * C],
                                     lhsT=x_sb[:, isl], rhs=wv_sb[:, :],
                                     start=True, stop=True)
                nc.vector.tensor_copy(
                    vt_sb[:, vb * VB:(vb + 1) * VB, 0:C],
                    pv[:, 0:VB * C].rearrange("p (v c) -> p v c", v=VB))

            def _compute():
                n_grp = (NI + GRP - 1) // GRP
                qk_done = 0
                vt_done = 0
                gidx = [0]         # global group counter (qk ring parity)
                step = [0]
                pend_av = []       # FIFO of (av, att, g, nb, js)
                pend_ep = []       # (av, js)

                def qk_tile():
                    # fp32 logits: TRN2 matmul can only write fp32 PSUM
                    # (16-bit PSUM output is TRN3-only), which pins the exp
                    # ring at 2x3 banks and GRP at 3
                    t = pp.tile([IB, GRP * JT], F32,
                                tag=("qkA" if gidx[0] % 2 == 0 else "qkB"),
                                name="qk")
                    gidx[0] += 1
                    return t

                def av_tile(jt):
                    return pp.tile([IB, JT], F32,
                                   tag=("avA" if jt % 2 == 0 else "avB"),
                                   name="av")

                def proj_tile():
                    # j-tile 0 projection scratch shares avB (av(jt1) is the
                    # next user of that bank, long after the last projection)
                    return pp.tile([IB, JT], F32, tag="avB", name="proj")

                def proj_tileA():
                    # chunk 0's k projection rides the avA bank, which is
                    # free until av(jt0)'s first accumulation at step 2 --
                    # this keeps it off pq(0)'s WAR chain so the first QK
                    # group isn't serialized behind two evacuations
                    return pp.tile([IB, JT], F32, tag="avA", name="projA")

                def ensure_vt(hi_block):
                    nonlocal vt_done
                    while vt_done * VB < hi_block:
                        emit_vt_proj(vt_done, proj_tile)
                        vt_done += 1

                def flush_av():
                    pav, patt, pg, pnb, pjs = pend_av.pop(0)
                    # one-group vt lookahead so the DVE evacuation is queued
                    # well before the AV matmuls that read it
                    ensure_vt(min(NI, pg * GRP + pnb + GRP))
                    for bi in range(pnb):
                        ib = pg * GRP + bi
                        nc.tensor.matmul(
                            pav[0:C + 1, :],
                            lhsT=vt_sb[:, ib, :],
                            rhs=patt[:, bi * JT:(bi + 1) * JT],
                            start=(ib == 0), stop=(ib == NI - 1))
                    if pg * GRP + pnb == NI:
                        pend_ep.append((pav, pjs))

                def flush_ep(final=False):
                    while pend_ep:
                        pav, pjs = pend_ep.pop(0)
                        # reciprocal straight off the PSUM denominator row
                        # (no den evacuation copy); the next j-tile
                        # accumulates into the other av bank, so the PE never
                        # stalls on this epilogue
                        if final:
                            # tail path: the qk ring is idle now, so a ones-
                            # stationary matmul broadcasts the reciprocal into
                            # a ring bank in ~0.5 us (the DMA broadcast below
                            # costs ~6.5 us of descriptor latency, hidden
                            # mid-run but fully exposed at the tail).  The
                            # 3.3 us DVE reciprocal is the tail's long pole,
                            # so run the epilogue in two 256-column halves:
                            # half 1's bcast/mult/add/DMA overlap half 2's
                            # reciprocal.
                            recip_r = wp.tile([IB, JT], F32R, tag="recipr")
                            bcps = pp.tile([IB, JT], F32, tag="qkA",
                                           name="bcps")
                            HJ = JT // 2
                            for h in range(2):
                                hs = slice(h * HJ, (h + 1) * HJ)
                                with nc.allow_low_precision(
                                        reason="f32r round of softmax recip"):
                                    nc.vector.reciprocal(recip_r[64:65, hs],
                                                         pav[64:65, hs])
                                nc.tensor.matmul(bcps[0:C, hs],
                                                 lhsT=ones_rows[64:65, 0:C],
                                                 rhs=recip_r[64:65, hs],
                                                 start=True, stop=True,
                                                 tile_position=(64, 0))
                            for h in range(2):
                                hs = slice(h * HJ, (h + 1) * HJ)
                                hjs = slice(pjs.start + h * HJ,
                                            pjs.start + (h + 1) * HJ)
                                bc_sb = wp.tile([C, HJ], F32, tag="bch")
                                nc.vector.tensor_copy(bc_sb[:, :],
                                                      bcps[0:C, hs])
                                oh = wp.tile([C, HJ], F32, tag="oh")
                                nc.vector.tensor_tensor(
                                    oh[:, :], pav[0:C, hs], bc_sb[:, :],
                                    op=mybir.AluOpType.mult)
                                nc.vector.tensor_tensor(
                                    oh[:, :], oh[:, :], x_raw[:, hjs],
                                    op=mybir.AluOpType.add)
                                nc.sync.dma_start(out=out_d[:, hjs],
                                                  in_=oh[:, :])
                            continue
                        else:
                            recip = wp.tile([IB, JT], F32, tag="recip")
                            nc.vector.reciprocal(recip[64:65, :],
                                                 pav[64:65, :])
                            bc_sb = wp.tile([C, JT], F32, tag="bc")
                            # partition-broadcast DMA: zero-step FREE dim on
                            # the src (the same 2 KB row read 64 times); a
                            # zero-step PARTITION dim is rejected by the DMA
                            # lowering.  ~6.5 us of completion latency, fully
                            # hidden by the next j-tile's compute
                            nc.sync.dma_start(
                                out=bc_sb[:, :],
                                in_=recip[64:65, None, :]
                                .to_broadcast((1, C, JT)))
                            bc_src = bc_sb[:, :]
                        o = wp.tile([C, JT], F32, tag="o")
                        nc.vector.tensor_tensor(o[:, :], pav[0:C, :],
                                                bc_src,
                                                op=mybir.AluOpType.mult)
                        nc.vector.tensor_tensor(o[:, :], o[:, :], x_raw[:, pjs],
                                                op=mybir.AluOpType.add)
                        # SWDGE (gpsimd) issue: that queue is idle mid-run
                        # and nothing waits on the store until the final
                        # drain, so its multi-us latency is free -- while
                        # the sync HWDGE queue stays dedicated to the
                        # broadcast DMA (sharing made the broadcast complete
                        # ~10 us late, thinning av-bank margins)
                        nc.gpsimd.dma_start(out=out_d[:, pjs], in_=o[:, :])

                # prologue: chunk-0 projections precede av(jt0)'s allocation
                # in both bank rings
                emit_proj(0, wq_rep, q_sb, proj_tile)
                emit_proj(0, wk_rep, k_sb, proj_tileA)
                qk_done = 1
                k_done = 1

                for jt in range(NJ):
                    js = slice(jt * JT, (jt + 1) * JT)
                    av = av_tile(jt)
                    for g in range(n_grp):
                        nb = min(GRP, NI - g * GRP)
                        if jt == 0:
                            # just-in-time q projections, one chunk ahead of
                            # the QK front; k chunk c is only read from
                            # j-tile c on, so the pk projections trickle one
                            # per group (halving early proj-bank WAR stalls)
                            hi_i = (g * GRP + nb) * IB
                            need = min(8, max(1, -(-hi_i // JT)) + 1)
                            while qk_done < need:
                                emit_proj(qk_done, wq_rep, q_sb, proj_tile)
                                qk_done += 1
                            if k_done < 8 and g >= 1:
                                emit_proj(k_done, wk_rep, k_sb, proj_tile,
                                          on_scalar=True)
                                k_done += 1
                            if g == n_grp - 1:
                                while qk_done < 8:
                                    emit_proj(qk_done, wq_rep, q_sb,
                                              proj_tile)
                                    qk_done += 1
                                while k_done < 8:
                                    emit_proj(k_done, wk_rep, k_sb,
                                              proj_tile, on_scalar=True)
                                    k_done += 1
                                # guarantee every avB-bank projection tile is
                                # emitted before av(jt1)'s allocation (the
                                # lazy flush_av path already reaches 32 here)
                                ensure_vt(NI)
                        qk = qk_tile()
                        for bi in range(nb):
                            ib = g * GRP + bi
                            isl = slice(ib * IB, (ib + 1) * IB)
                            nc.tensor.matmul(
                                qk[:, bi * JT:(bi + 1) * JT],
                                lhsT=q_sb[32 * bi:32 * bi + D, isl],
                                rhs=k_sb[32 * bi:32 * bi + D, js],
                                start=True, stop=True,
                                tile_position=(32 * bi, 0))
                        att = wp.tile([IB, GRP * JT], BF16, tag="att")
                        nc.scalar.activation(
                            att[:, 0:nb * JT], qk[:, 0:nb * JT],
                            mybir.ActivationFunctionType.Exp)
                        flush_ep()
                        pend_av.append((av, att, g, nb, js))
                        while len(pend_av) > AV_LAG:
                            flush_av()
                        step[0] += 1
                while pend_av:
                    flush_av()
                    flush_ep(final=True)
                flush_ep(final=True)

            if loop_n:
                hints = (mybir.EngineType.PE, mybir.EngineType.Activation,
                         mybir.EngineType.DVE, mybir.EngineType.SP,
                         mybir.EngineType.Pool)
                with tc.For_i(0, loop_n, 1, hint_engines=hints):
                    for _ in range(bodies):
                        x_rounded[:] = [False] * 8
                        _compute()
            else:
                _compute()

    _fix_drain_waits(nc)
    return nc


_NC_CACHE = {}


def _get_nc():
    if "nc" not in _NC_CACHE:
        _NC_CACHE["nc"] = build_nc()
    return _NC_CACHE["nc"]


def kernel(**inputs) -> np.ndarray:
    x = np.ascontiguousarray(np.asarray(inputs["x"], dtype=np.float32))
    assert x.shape == (B, C, H, W), x.shape
    weights = {
        name: np.ascontiguousarray(np.asarray(inputs[name], dtype=np.float32))
        for name in ("Wq", "bq", "Wk", "bk", "Wv", "bv")
    }
    in_maps = [{"x": x[b].reshape(C, HW), **weights} for b in range(B)]
    nc = _get_nc()
    res = run_bass_kernel_spmd(nc, in_maps, core_ids=list(range(B)))
    out = np.stack([np.asarray(res.results[b]["out"]).reshape(C, H, W)
                    for b in range(B)])
    return out.astype(np.float32)


# revision 58
# speedup vs baseline: 1.2153x; 1.1363x over previous
"""CrissCross(actually full)-attention Trainium2 kernel.

Reference computation per batch b (C=64 channels, HW=4096 positions, D=8):
    q = Wq@x + bq        [D, HW]
    k = Wk@x + bk        [D, HW]
    v = Wv@x + bv        [C, HW]
    att[i, j] = softmax_i(q[:, i] . k[:, j])
    out[c, j] = sum_i v[c, i] att[i, j] + x[c, j]

Sharding: data-parallel, one batch per NeuronCore (8 cores).

Measured HW model (from NTFF traces of this kernel's runs):
  - QK group (3 row-tiled [8,128]x[8,512] f32r matmuls) is STREAM-bound:
    wall ~727 ns cold (K=4/8, 1.2 GHz) / ~435 ns warm (K=8/8, 2.4 GHz).
  - AV matmul ([128,65]bf16 stationary, 512-col stream): cadence 427 cold /
    241 warm.
  - ScalarE exp of [128, 1536]: 1573 ns cold-phase, 1423 measured in steady
    warm state (ScalarE ~1.33 GHz effective); 88 of them set the ~125-138 us
    ScalarE floor.  Per group: PE cold 2008 ns (PE-paced); PE warm 1158 <
    exp (ScalarE-paced).  Exps are irreducible: 16.7M per core on the only
    exp-capable engine.
  - PE_HAM: the un-throttle grant (K=4/8 -> 8/8) arrives 60-140 us into the
    run at a firmware-paced, effectively random time; early dummy-matmul
    warmup bursts do NOT move it (measured), and grant "blips" die if a PE
    stall lands inside the 3.4 us grant window -- hence the all-out war on
    pipeline gaps below.  Under sustained board heat a separate P0 state
    downclocks PE 2.4->2.0 and ScalarE ~1.33->1.1, inflating everything
    ~15-20%; run-to-run comparisons must account for it.

Per-core dataflow: x'=[x;ones] (biases folded via the ones row); q and k are
projected chunk-by-chunk with REPLICATED stationaries (weight columns at
{0-7,32-39,64-71}) so each 512-wide chunk lands with its row-tiling replicas
in place and one [72,512] DVE copy evacuates it (never-read garbage rows
ride along).  k chunk c is first read in j-tile c, so pk projections trickle
one per group through j-tile 0 and evacuate on ScalarE's slack; vT' (bf16,
trailing ones column -> AV emits numerator and denominator together) is
projected 4 blocks per tile with one-group lookahead.  Softmax skips
max-subtraction (|logit| < ~26, well inside fp32/bf16 exp range).

PSUM layout (single pool, 8 banks, explicit tags):
  qkA [128,1536] banks 0-2   |  exp-input ring, alternating per group
  qkB [128,1536] banks 3-5   |  (global group parity across j-tiles)
  avA [128, 512] bank 6      |  AV accumulator, alternating per j-tile
  avB [128, 512] bank 7      |  parity; j-tile 0's projection scratch
                                also rotates through these banks
Epilogue per j-tile (runs while the next j-tile computes, PE-free):
  DVE reciprocal reads the denominator row (av[64]) straight from PSUM; a
  partition-broadcast DMA (zero-step FREE dim; ~6.5 us descriptor latency,
  fully hidden) replicates it to 64 rows; DVE mult (+x residual) and DMA
  out.  The next j-tile accumulates into the other av bank, so the PE never
  stalls on the epilogue -- stall-free j-tile boundaries are what let HAM
  warm windows survive once granted.  The LAST j-tile instead broadcasts
  via a ones-stationary matmul into the now-idle qk ring (tile_position
  (64,0)) in two 256-column halves, hiding half the 3.3 us reciprocal.
"""

import numpy as np

import bass_rust
import concourse.bass as bass
import concourse.tile as tile
from concourse import mybir
from concourse.bass_utils import run_bass_kernel_spmd

B, C, HW, D = 8, 64, 4096, 8
H = W = 64
JT = 512          # j-tile width (PSUM bank)
NJ = HW // JT     # 8
IB = 128          # i-block height (partitions)
NI = HW // IB     # 32
GRP = 3           # i-blocks per exp group (3-way row tiling)
N_WARM = 0        # boot HAM-warmup dummy matmuls: measured useless (the
                  # PE_HAM un-throttle grant is firmware-paced, ~85-127 us
                  # into the run regardless of early PE activity)
VB = 4            # vT i-blocks projected per PSUM tile
AV_LAG = 2        # groups the AV flush trails the QK/exp front
TAIL_LAG = 2      # groups the PE bcast trails the epilogue's reciprocal

F32 = mybir.dt.float32
F32R = mybir.dt.float32r
F16 = mybir.dt.float16
BF16 = mybir.dt.bfloat16


def _fix_drain_waits(nc):
    """walrus in this container rejects instructions carrying more than one
    sync-wait; hoist extras onto NoOps inserted just before, same engine."""
    for f in nc.m.functions:
        for blk in f.blocks:
            insts = blk.instructions
            for tgt in [
                i for i in list(insts)
                if i.sync_info and len(i.sync_info.on_wait or []) > 1
            ]:
                si = tgt.sync_info
                waits = list(si.on_wait)
                si.on_wait = waits[-1:]
                di = insts.index(tgt)
                for w in waits[:-1]:
                    n = nc.engines[tgt.engine].nop()
                    for b in f.blocks:
                        bi = b.instructions
                        for idx in range(len(bi) - 1, -1, -1):
                            if bi[idx].name == n.ins.name:
                                bi.pop(idx)
                                break
                    n.ins.sync_info = bass_rust.SyncInfo(on_wait=[w], on_update=[])
                    insts.insert(di, n.ins)
                    di += 1


def build_nc(loop_n=None, bodies=1):
    nc = bass.Bass()
    x_d = nc.dram_tensor("x", [C, HW], F32, kind="ExternalInput")
    wq_d = nc.dram_tensor("Wq", [D, C], F32, kind="ExternalInput")
    bq_d = nc.dram_tensor("bq", [D], F32, kind="ExternalInput")
    wk_d = nc.dram_tensor("Wk", [D, C], F32, kind="ExternalInput")
    bk_d = nc.dram_tensor("bk", [D], F32, kind="ExternalInput")
    wv_d = nc.dram_tensor("Wv", [C, C], F32, kind="ExternalInput")
    bv_d = nc.dram_tensor("bv", [C], F32, kind="ExternalInput")
    out_d = nc.dram_tensor("out", [C, HW], F32, kind="ExternalOutput")

    with tile.TileContext(nc) as tc:
        with (
            tc.tile_pool(name="const", bufs=1) as cp,
            tc.tile_pool(name="work", bufs=4) as wp,
            tc.tile_pool(name="qtmp", bufs=2) as qp,
            tc.tile_pool(name="ps", bufs=1, space="PSUM") as pp,
        ):
            # ---- persistent SBUF tensors ----
            x_raw = cp.tile([C, HW], F32, tag="xraw")        # residual source
            x_sb = cp.tile([C + 1, HW], F32R, tag="x")       # x' = [x; ones]
            # raw weights land CONTIGUOUS and are transposed on-chip (DVE
            # 32x32 block transposes); transposing DMAs cost us of descriptors
            wqwk_raw = cp.tile([32, 2 * C], F32, tag="wqwkraw")
            wv_raw = cp.tile([C, C], F32, tag="wvraw")
            bias_raw = cp.tile([1, 2 * D + C], F32, tag="braw")
            wqkT = cp.tile([C, C], F32, tag="wqkT")            # [WqT | WkT]
            wvT = cp.tile([C, C], F32, tag="wvT")
            # replicated projection stationaries: weight columns at
            # {0-7, 32-39, 64-71} so the projection matmul emits q (resp. k)
            # with its row-tiling replicas already in place -- one [72,512]
            # DVE copy evacuates a whole chunk (cols 8-31/40-63 are never
            # read downstream, so they stay uninitialized)
            wq_rep = cp.tile([C + 1, GRP, 32], F32R, tag="wqrep")
            wk_rep = cp.tile([C + 1, GRP, 32], F32R, tag="wkrep")
            wv_sb = cp.tile([C + 1, C], F32R, tag="wv")        # [WvT; bv]
            q_sb = cp.tile([64 + D, HW], F32R, tag="q")    # replicas @0/32/64
            k_sb = cp.tile([64 + D, HW], F32R, tag="k")
            vt_sb = cp.tile([IB, NI, C + 1], BF16, tag="vt")   # vT' blocks
            ones_sb = cp.tile([IB, 1], F32, tag="ones")
            # tail-epilogue bcast stationary (row 64) + HAM-warmup operands
            ones_rows = cp.tile([IB, C], F32R, tag="onesrow")
            warm_rhs = cp.tile([1, JT], F32R, tag="warmrhs")
            warm_sb = cp.tile([1, 4], F32, tag="warm")         # act-table warmup

            # ---- boot: critical DMAs first, then DVE chain in dep order ----
            # x chunk 0 + q/k weights gate the first projection; they go at
            # the head of their queues.  Engine queues are in-order, so the
            # emission order below IS the issue order.
            nc.vector.memset(wqwk_raw[:, :], 0.0)
            nc.sync.dma_start(out=x_raw[:, 0:JT], in_=x_d[:, 0:JT])
            nc.sync.dma_start(out=wqwk_raw[0:D, 0:C], in_=wq_d[:, :])
            nc.sync.dma_start(out=wqwk_raw[0:D, C:2 * C], in_=wk_d[:, :])
            nc.sync.dma_start(out=bias_raw[0:1, 0:D], in_=bq_d[None, :])
            nc.sync.dma_start(out=bias_raw[0:1, D:2 * D], in_=bk_d[None, :])
            nc.scalar.dma_start(out=x_raw[:, JT:2 * JT], in_=x_d[:, JT:2 * JT])
            nc.scalar.dma_start(out=wv_raw[:, :], in_=wv_d[:, :])
            nc.scalar.dma_start(out=bias_raw[0:1, 2 * D:], in_=bv_d[None, :])
            # all x chunks ride HWDGE queues (sync/scalar): SWDGE's multi-us
            # spin-up latency on the gpsimd queue was gating the j-tile-0
            # x-round CASTs, stalling the projection chain
            nc.sync.dma_start(out=x_raw[:, 2 * JT:3 * JT],
                              in_=x_d[:, 2 * JT:3 * JT])
            nc.scalar.dma_start(out=x_raw[:, 3 * JT:4 * JT],
                                in_=x_d[:, 3 * JT:4 * JT])
            nc.sync.dma_start(out=x_raw[:, 4 * JT:5 * JT],
                              in_=x_d[:, 4 * JT:5 * JT])
            nc.scalar.dma_start(out=x_raw[:, 5 * JT:6 * JT],
                                in_=x_d[:, 5 * JT:6 * JT])
            nc.sync.dma_start(out=x_raw[:, 6 * JT:7 * JT],
                              in_=x_d[:, 6 * JT:7 * JT])
            nc.scalar.dma_start(out=x_raw[:, 7 * JT:8 * JT],
                                in_=x_d[:, 7 * JT:8 * JT])
            # constants on GpSimd (keeps the DVE queue free for the boot
            # critical chain); wqk_sb zero covers the unused cols 8-31
            nc.gpsimd.memset(ones_sb[:, :], 1.0)
            nc.gpsimd.memset(vt_sb[:, :, C:C + 1], 1.0)
            nc.gpsimd.memset(ones_rows[:, :].bitcast(F32), 1.0)
            nc.gpsimd.memset(wq_rep[:, :, :].bitcast(F32), 0.0)
            nc.gpsimd.memset(wk_rep[:, :, :].bitcast(F32), 0.0)
            # pre-load the ScalarE activation table during the DMA wait
            nc.scalar.activation(warm_sb[0:1, :],
                                 ones_sb[0:1, 0:1].to_broadcast([1, 4]),
                                 mybir.ActivationFunctionType.Exp)
            # HAM warmup: PE_HAM un-throttles only after a long stretch of
            # busy activity windows; burn the otherwise-idle boot DMA wait on
            # dummy matmuls so the busy counter starts at ~1 us, not ~15 us
            if N_WARM:
                warm_ps = pp.tile([IB, GRP * JT], F32, tag="qkA",
                                  name="warmps")
                for i in range(N_WARM):
                    nc.tensor.matmul(warm_ps[0:C, 0:JT],
                                     lhsT=ones_rows[0:1, :],
                                     rhs=warm_rhs[0:1, :],
                                     start=(i == 0), stop=(i == N_WARM - 1))
            # on-chip transposes: Wq/Wk rows live in wqwk_raw[0:8] (rest
            # zeroed); DVE transposes 32x32 blocks
            for m in range(2):          # 0 = q, 1 = k
                for j in range(2):
                    nc.vector.transpose(
                        wqkT[32 * j:32 * j + 32, 32 * m:32 * m + 32],
                        wqwk_raw[0:32, m * C + 32 * j:m * C + 32 * j + 32])
            # free-dim-broadcast copies place each weight block at the three
            # 32-aligned column positions in one DVE instruction
            nc.vector.tensor_copy(
                wq_rep[0:C, :, 0:D],
                wqkT[0:C, None, 0:D].to_broadcast((C, GRP, D)))
            nc.vector.tensor_copy(
                wq_rep[C:C + 1, :, 0:D],
                bias_raw[0:1, None, 0:D].to_broadcast((1, GRP, D)))
            nc.vector.tensor_copy(
                wk_rep[0:C, :, 0:D],
                wqkT[0:C, None, 32:32 + D].to_broadcast((C, GRP, D)))
            nc.vector.tensor_copy(
                wk_rep[C:C + 1, :, 0:D],
                bias_raw[0:1, None, D:2 * D].to_broadcast((1, GRP, D)))

            x_rounded = [False] * 8
            wv_init = [False]

            def ensure_x(ch):
                """Round x chunk ch (512 wide) to f32r lazily so the startup
                chain doesn't queue behind the whole x preprocessing."""
                cs = slice(ch * JT, (ch + 1) * JT)
                if x_rounded[ch]:
                    return
                x_rounded[ch] = True
                nc.vector.tensor_copy(x_sb[0:C, cs], x_raw[:, cs])
                nc.gpsimd.memset(x_sb[C:C + 1, cs].bitcast(F32), 1.0)

            def emit_proj(ct, w_rep, dst, proj_tile, on_scalar=False):
                """Project one tensor (q or k) for HW-chunk ct (512 wide);
                the replicated stationary lands it at partition groups
                {0, 32, 64} directly, so one [72,512] DVE copy evacuates the
                chunk (garbage rows 8-31/40-63 go along for the ride into
                never-read q_sb/k_sb rows)."""
                ensure_x(ct)
                js = slice(ct * JT, (ct + 1) * JT)
                p = proj_tile()
                nc.tensor.matmul(p[0:64 + D, :],
                                 lhsT=w_rep[:, :, :].rearrange(
                                     "p g c -> p (g c)")[:, 0:64 + D],
                                 rhs=x_sb[:, js], start=True, stop=True)
                if on_scalar:
                    # k chunks c>=1 are first read in j-tile c; their
                    # evacuation rides ScalarE's jt0 slack instead of the
                    # DVE queue (which gates the q/vt critical chains)
                    nc.scalar.activation(dst[:, js], p[0:64 + D, :],
                                         mybir.ActivationFunctionType.Copy)
                else:
                    nc.vector.tensor_copy(dst[:, js], p[0:64 + D, :])

            def emit_vt_proj(vb, proj_tile):
                """Project vT' i-blocks vb*VB .. vb*VB+VB-1."""
                if not wv_init[0]:
                    wv_init[0] = True
                    for i in range(2):
                        for j in range(2):
                            nc.vector.transpose(
                                wvT[32 * j:32 * j + 32, 32 * i:32 * i + 32],
                                wv_raw[32 * i:32 * i + 32, 32 * j:32 * j + 32])
                    nc.vector.tensor_copy(wv_sb[0:C, :], wvT[:, :])
                    nc.vector.tensor_copy(wv_sb[C:C + 1, :],
                                          bias_raw[0:1, 2 * D:])
                ensure_x((vb * VB * IB) // JT)
                ensure_x(((vb + 1) * VB * IB - 1) // JT)
                pv = proj_tile()
                for u in range(VB):
                    ib = vb * VB + u
                    isl = slice(ib * IB, (ib + 1) * IB)
                    nc.tensor.matmul(pv[0:IB, u * C:(u + 1) * C],
                                     lhsT=x_sb[:, isl], rhs=wv_sb[:, :],
                                     start=True, stop=True)
                nc.vector.tensor_copy(
                    vt_sb[:, vb * VB:(vb + 1) * VB, 0:C],
                    pv[:, 0:VB * C].rearrange("p (v c) -> p v c", v=VB))

            def _compute():
                n_grp = (NI + GRP - 1) // GRP
                qk_done = 0
                vt_done = 0
                gidx = [0]         # global group counter (qk ring parity)
                step = [0]
                pend_av = []       # FIFO of (av, att, g, nb, js)
                pend_ep = []       # (av, js)

                def qk_tile():
                    # fp32 logits: TRN2 matmul can only write fp32 PSUM
                    # (16-bit PSUM output is TRN3-only), which pins the exp
                    # ring at 2x3 banks and GRP at 3
                    t = pp.tile([IB, GRP * JT], F32,
                                tag=("qkA" if gidx[0] % 2 == 0 else "qkB"),
                                name="qk")
                    gidx[0] += 1
                    return t

                def av_tile(jt):
                    return pp.tile([IB, JT], F32,
                                   tag=("avA" if jt % 2 == 0 else "avB"),
                                   name="av")

                def proj_tile():
                    # j-tile 0 projection scratch shares avB (av(jt1) is the
                    # next user of that bank, long after the last projection)
                    return pp.tile([IB, JT], F32, tag="avB", name="proj")

                def proj_tileA():
                    # chunk 0's k projection rides the avA bank, which is
                    # free until av(jt0)'s first accumulation at step 2 --
                    # this keeps it off pq(0)'s WAR chain so the first QK
                    # group isn't serialized behind two evacuations
                    return pp.tile([IB, JT], F32, tag="avA", name="projA")

                def ensure_vt(hi_block):
                    nonlocal vt_done
                    while vt_done * VB < hi_block:
                        emit_vt_proj(vt_done, proj_tile)
                        vt_done += 1

                def flush_av():
                    pav, patt, pg, pnb, pjs = pend_av.pop(0)
                    # one-group vt lookahead so the DVE evacuation is queued
                    # well before the AV matmuls that read it
                    ensure_vt(min(NI, pg * GRP + pnb + GRP))
                    for bi in range(pnb):
                        ib = pg * GRP + bi
                        nc.tensor.matmul(
                            pav[0:C + 1, :],
                            lhsT=vt_sb[:, ib, :],
                            rhs=patt[:, bi * JT:(bi + 1) * JT],
                            start=(ib == 0), stop=(ib == NI - 1))
                    if pg * GRP + pnb == NI:
                        pend_ep.append((pav, pjs))

                def flush_ep(final=False):
                    while pend_ep:
                        pav, pjs = pend_ep.pop(0)
                        # reciprocal straight off the PSUM denominator row
                        # (no den evacuation copy); the next j-tile
                        # accumulates into the other av bank, so the PE never
                        # stalls on this epilogue
                        if final:
                            # tail path: the qk ring is idle now, so a ones-
                            # stationary matmul broadcasts the reciprocal into
                            # a ring bank in ~0.5 us (the DMA broadcast below
                            # costs ~6.5 us of descriptor latency, hidden
                            # mid-run but fully exposed at the tail).  The
                            # 3.3 us DVE reciprocal is the tail's long pole,
                            # so run the epilogue in two 256-column halves:
                            # half 1's bcast/mult/add/DMA overlap half 2's
                            # reciprocal.
                            recip_r = wp.tile([IB, JT], F32R, tag="recipr")
                            bcps = pp.tile([IB, JT], F32, tag="qkA",
                                           name="bcps")
                            HJ = JT // 2
                            for h in range(2):
                                hs = slice(h * HJ, (h + 1) * HJ)
                                with nc.allow_low_precision(
                                        reason="f32r round of softmax recip"):
                                    nc.vector.reciprocal(recip_r[64:65, hs],
                                                         pav[64:65, hs])
                                nc.tensor.matmul(bcps[0:C, hs],
                                                 lhsT=ones_rows[64:65, 0:C],
                                                 rhs=recip_r[64:65, hs],
                                                 start=True, stop=True,
                                                 tile_position=(64, 0))
                            for h in range(2):
                                hs = slice(h * HJ, (h + 1) * HJ)
                                hjs = slice(pjs.start + h * HJ,
                                            pjs.start + (h + 1) * HJ)
                                bc_sb = wp.tile([C, HJ], F32, tag="bch")
                                nc.vector.tensor_copy(bc_sb[:, :],
                                                      bcps[0:C, hs])
                                oh = wp.tile([C, HJ], F32, tag="oh")
                                nc.vector.tensor_tensor(
                                    oh[:, :], pav[0:C, hs], bc_sb[:, :],
                                    op=mybir.AluOpType.mult)
                                nc.vector.tensor_tensor(
                                    oh[:, :], oh[:, :], x_raw[:, hjs],
                                    op=mybir.AluOpType.add)
                                nc.sync.dma_start(out=out_d[:, hjs],
                                                  in_=oh[:, :])
                            continue
                        else:
                            recip = wp.tile([IB, JT], F32, tag="recip")
                            nc.vector.reciprocal(recip[64:65, :],
                                                 pav[64:65, :])
                            bc_sb = wp.tile([C, JT], F32, tag="bc")
                            # partition-broadcast DMA: zero-step FREE dim on
                            # the src (the same 2 KB row read 64 times); a
                            # zero-step PARTITION dim is rejected by the DMA
                            # lowering.  ~6.5 us of completion latency, fully
                            # hidden by the next j-tile's compute
                            nc.sync.dma_start(
                                out=bc_sb[:, :],
                                in_=recip[64:65, None, :]
                                .to_broadcast((1, C, JT)))
                            bc_src = bc_sb[:, :]
                        o = wp.tile([C, JT], F32, tag="o")
                        nc.vector.tensor_tensor(o[:, :], pav[0:C, :],
                                                bc_src,
                                                op=mybir.AluOpType.mult)
                        nc.vector.tensor_tensor(o[:, :], o[:, :], x_raw[:, pjs],
                                                op=mybir.AluOpType.add)
                        # SWDGE (gpsimd) issue: that queue is idle mid-run
                        # and nothing waits on the store until the final
                        # drain, so its multi-us latency is free -- while
                        # the sync HWDGE queue stays dedicated to the
                        # broadcast DMA (sharing made the broadcast complete
                        # ~10 us late, thinning av-bank margins)
                        nc.gpsimd.dma_start(out=out_d[:, pjs], in_=o[:, :])

                # prologue: chunk-0 projections precede av(jt0)'s allocation
                # in both bank rings
                emit_proj(0, wq_rep, q_sb, proj_tile)
                emit_proj(0, wk_rep, k_sb, proj_tileA)
                qk_done = 1
                k_done = 1

                for jt in range(NJ):
                    js = slice(jt * JT, (jt + 1) * JT)
                    av = av_tile(jt)
                    for g in range(n_grp):
                        nb = min(GRP, NI - g * GRP)
                        if jt == 0:
                            # round x chunks two groups ahead of the proj
                            # front: the x-round CAST carries a coarse WAR
                            # wait on PE progress (Tile tracks x_sb at tensor
                            # granularity), which stalled the next projection
                            # by ~500-800 ns when emitted just-in-time
                            for ch in range(min(8, g + 3)):
                                ensure_x(ch)
                            # just-in-time q projections, one chunk ahead of
                            # the QK front; k chunk c is only read from
                            # j-tile c on, so the pk projections trickle one
                            # per group (halving early proj-bank WAR stalls)
                            hi_i = (g * GRP + nb) * IB
                            need = min(8, max(1, -(-hi_i // JT)) + 1)
                            while qk_done < need:
                                emit_proj(qk_done, wq_rep, q_sb, proj_tile)
                                qk_done += 1
                            if k_done < 8 and g >= 1:
                                emit_proj(k_done, wk_rep, k_sb, proj_tile,
                                          on_scalar=True)
                                k_done += 1
                            if g == n_grp - 1:
                                while qk_done < 8:
                                    emit_proj(qk_done, wq_rep, q_sb,
                                              proj_tile)
                                    qk_done += 1
                                while k_done < 8:
                                    emit_proj(k_done, wk_rep, k_sb,
                                              proj_tile, on_scalar=True)
                                    k_done += 1
                                # guarantee every avB-bank projection tile is
                                # emitted before av(jt1)'s allocation (the
                                # lazy flush_av path already reaches 32 here)
                                ensure_vt(NI)
                        qk = qk_tile()
                        for bi in range(nb):
                            ib = g * GRP + bi
                            isl = slice(ib * IB, (ib + 1) * IB)
                            nc.tensor.matmul(
                                qk[:, bi * JT:(bi + 1) * JT],
                                lhsT=q_sb[32 * bi:32 * bi + D, isl],
                                rhs=k_sb[32 * bi:32 * bi + D, js],
                                start=True, stop=True,
                                tile_position=(32 * bi, 0))
                        att = wp.tile([IB, GRP * JT], BF16, tag="att")
                        nc.scalar.activation(
                            att[:, 0:nb * JT], qk[:, 0:nb * JT],
                            mybir.ActivationFunctionType.Exp)
                        flush_ep()
                        pend_av.append((av, att, g, nb, js))
                        while len(pend_av) > AV_LAG:
                            flush_av()
                        step[0] += 1
                while pend_av:
                    flush_av()
                    flush_ep(final=True)
                flush_ep(final=True)

            if loop_n:
                hints = (mybir.EngineType.PE, mybir.EngineType.Activation,
                         mybir.EngineType.DVE, mybir.EngineType.SP,
                         mybir.EngineType.Pool)
                with tc.For_i(0, loop_n, 1, hint_engines=hints):
                    for _ in range(bodies):
                        x_rounded[:] = [False] * 8
                        _compute()
            else:
                _compute()

    _fix_drain_waits(nc)
    return nc


_NC_CACHE = {}


def _get_nc():
    if "nc" not in _NC_CACHE:
        _NC_CACHE["nc"] = build_nc()
    return _NC_CACHE["nc"]


def kernel(**inputs) -> np.ndarray:
    x = np.ascontiguousarray(np.asarray(inputs["x"], dtype=np.float32))
    assert x.shape == (B, C, H, W), x.shape
    weights = {
        name: np.ascontiguousarray(np.asarray(inputs[name], dtype=np.float32))
        for name in ("Wq", "bq", "Wk", "bk", "Wv", "bv")
    }
    in_maps = [{"x": x[b].reshape(C, HW), **weights} for b in range(B)]
    nc = _get_nc()
    res = run_bass_kernel_spmd(nc, in_maps, core_ids=list(range(B)))
    out = np.stack([np.asarray(res.results[b]["out"]).reshape(C, H, W)
                    for b in range(B)])
    return out.astype(np.float32)


# revision 59
# speedup vs baseline: 1.3183x; 1.0847x over previous
"""CrissCross(actually full)-attention Trainium2 kernel.

Reference computation per batch b (C=64 channels, HW=4096 positions, D=8):
    q = Wq@x + bq        [D, HW]
    k = Wk@x + bk        [D, HW]
    v = Wv@x + bv        [C, HW]
    att[i, j] = softmax_i(q[:, i] . k[:, j])
    out[c, j] = sum_i v[c, i] att[i, j] + x[c, j]

Sharding: data-parallel, one batch per NeuronCore (8 cores).

Measured HW model (from NTFF traces of this kernel's runs):
  - QK group (3 row-tiled [8,128]x[8,512] f32r matmuls) is STREAM-bound:
    wall ~727 ns cold (K=4/8, 1.2 GHz) / ~435 ns warm (K=8/8, 2.4 GHz).
  - AV matmul ([128,65]bf16 stationary, 512-col stream): cadence 427 cold /
    241 warm.
  - ScalarE exp of [128, 1536]: 1573 ns cold-phase, 1423 measured in steady
    warm state (ScalarE ~1.33 GHz effective); 88 of them set the ~125-138 us
    ScalarE floor.  Per group: PE cold 2008 ns (PE-paced); PE warm 1158 <
    exp (ScalarE-paced).  Exps are irreducible: 16.7M per core on the only
    exp-capable engine.
  - PE_HAM: the un-throttle grant (K=4/8 -> 8/8) arrives 60-140 us into the
    run at a firmware-paced, effectively random time; early dummy-matmul
    warmup bursts do NOT move it (measured), and grant "blips" die if a PE
    stall lands inside the 3.4 us grant window -- hence the all-out war on
    pipeline gaps below.  Under sustained board heat a separate P0 state
    downclocks PE 2.4->2.0 and ScalarE ~1.33->1.1, inflating everything
    ~15-20%; run-to-run comparisons must account for it.

Per-core dataflow: x'=[x;ones] (biases folded via the ones row); q and k are
projected chunk-by-chunk with REPLICATED stationaries (weight columns at
{0-7,32-39,64-71}) so each 512-wide chunk lands with its row-tiling replicas
in place and one [72,512] DVE copy evacuates it (never-read garbage rows
ride along).  k chunk c is first read in j-tile c, so pk projections trickle
one per group through j-tile 0 and evacuate on ScalarE's slack; vT' (bf16,
trailing ones column -> AV emits numerator and denominator together) is
projected 4 blocks per tile with one-group lookahead.  Softmax skips
max-subtraction (|logit| < ~26, well inside fp32/bf16 exp range).

PSUM layout (single pool, 8 banks, explicit tags):
  qkA [128,1536] banks 0-2   |  exp-input ring, alternating per group
  qkB [128,1536] banks 3-5   |  (global group parity across j-tiles)
  avA [128, 512] bank 6      |  AV accumulator, alternating per j-tile
  avB [128, 512] bank 7      |  parity; j-tile 0's projection scratch
                                also rotates through these banks
Epilogue per j-tile (runs while the next j-tile computes, PE-free):
  DVE reciprocal reads the denominator row (av[64]) straight from PSUM; a
  partition-broadcast DMA (zero-step FREE dim; ~6.5 us descriptor latency,
  fully hidden) replicates it to 64 rows; DVE mult (+x residual) and DMA
  out.  The next j-tile accumulates into the other av bank, so the PE never
  stalls on the epilogue -- stall-free j-tile boundaries are what let HAM
  warm windows survive once granted.  The LAST j-tile instead broadcasts
  via a ones-stationary matmul into the now-idle qk ring (tile_position
  (64,0)) in two 256-column halves, hiding half the 3.3 us reciprocal.
"""

import numpy as np

import bass_rust
import concourse.bass as bass
import concourse.tile as tile
from concourse import mybir
from concourse.bass_utils import run_bass_kernel_spmd

B, C, HW, D = 8, 64, 4096, 8
H = W = 64
JT = 512          # j-tile width (PSUM bank)
NJ = HW // JT     # 8
IB = 128          # i-block height (partitions)
NI = HW // IB     # 32
GRP = 3           # i-blocks per exp group (3-way row tiling)
N_WARM = 0        # boot HAM-warmup dummy matmuls: measured useless (the
                  # PE_HAM un-throttle grant is firmware-paced, ~85-127 us
                  # into the run regardless of early PE activity)
VB = 4            # vT i-blocks projected per PSUM tile
AV_LAG = 2        # groups the AV flush trails the QK/exp front
TAIL_LAG = 2      # groups the PE bcast trails the epilogue's reciprocal

F32 = mybir.dt.float32
F32R = mybir.dt.float32r
F16 = mybir.dt.float16
BF16 = mybir.dt.bfloat16


def _fix_drain_waits(nc):
    """walrus in this container rejects instructions carrying more than one
    sync-wait; hoist extras onto NoOps inserted just before, same engine."""
    for f in nc.m.functions:
        for blk in f.blocks:
            insts = blk.instructions
            for tgt in [
                i for i in list(insts)
                if i.sync_info and len(i.sync_info.on_wait or []) > 1
            ]:
                si = tgt.sync_info
                waits = list(si.on_wait)
                si.on_wait = waits[-1:]
                di = insts.index(tgt)
                for w in waits[:-1]:
                    n = nc.engines[tgt.engine].nop()
                    for b in f.blocks:
                        bi = b.instructions
                        for idx in range(len(bi) - 1, -1, -1):
                            if bi[idx].name == n.ins.name:
                                bi.pop(idx)
                                break
                    n.ins.sync_info = bass_rust.SyncInfo(on_wait=[w], on_update=[])
                    insts.insert(di, n.ins)
                    di += 1


def build_nc(loop_n=None, bodies=1):
    nc = bass.Bass()
    x_d = nc.dram_tensor("x", [C, HW], F32, kind="ExternalInput")
    wq_d = nc.dram_tensor("Wq", [D, C], F32, kind="ExternalInput")
    bq_d = nc.dram_tensor("bq", [D], F32, kind="ExternalInput")
    wk_d = nc.dram_tensor("Wk", [D, C], F32, kind="ExternalInput")
    bk_d = nc.dram_tensor("bk", [D], F32, kind="ExternalInput")
    wv_d = nc.dram_tensor("Wv", [C, C], F32, kind="ExternalInput")
    bv_d = nc.dram_tensor("bv", [C], F32, kind="ExternalInput")
    out_d = nc.dram_tensor("out", [C, HW], F32, kind="ExternalOutput")

    with tile.TileContext(nc) as tc:
        with (
            tc.tile_pool(name="const", bufs=1) as cp,
            tc.tile_pool(name="work", bufs=4) as wp,
            tc.tile_pool(name="qtmp", bufs=2) as qp,
            tc.tile_pool(name="ps", bufs=1, space="PSUM") as pp,
        ):
            # ---- persistent SBUF tensors ----
            x_raw = cp.tile([C, HW], F32, tag="xraw")        # residual source
            x_sb = cp.tile([C + 1, HW], F32R, tag="x")       # x' = [x; ones]
            # raw weights land CONTIGUOUS and are transposed on-chip (DVE
            # 32x32 block transposes); transposing DMAs cost us of descriptors
            wqwk_raw = cp.tile([32, 2 * C], F32, tag="wqwkraw")
            wv_raw = cp.tile([C, C], F32, tag="wvraw")
            bias_raw = cp.tile([1, 2 * D + C], F32, tag="braw")
            wqkT = cp.tile([C, C], F32, tag="wqkT")            # [WqT | WkT]
            wvT = cp.tile([C, C], F32, tag="wvT")
            # replicated projection stationaries: weight columns at
            # {0-7, 32-39, 64-71} so the projection matmul emits q (resp. k)
            # with its row-tiling replicas already in place -- one [72,512]
            # DVE copy evacuates a whole chunk (cols 8-31/40-63 are never
            # read downstream, so they stay uninitialized)
            wq_rep = cp.tile([C + 1, GRP, 32], F32R, tag="wqrep")
            wk_rep = cp.tile([C + 1, GRP, 32], F32R, tag="wkrep")
            wv_sb = cp.tile([C + 1, C], F32R, tag="wv")        # [WvT; bv]
            q_sb = cp.tile([64 + D, HW], F32R, tag="q")    # replicas @0/32/64
            k_sb = cp.tile([64 + D, HW], F32R, tag="k")
            vt_sb = cp.tile([IB, NI, C + 1], BF16, tag="vt")   # vT' blocks
            ones_sb = cp.tile([IB, 1], F32, tag="ones")
            # tail-epilogue bcast stationary (row 64) + HAM-warmup operands
            ones_rows = cp.tile([IB, C], F32R, tag="onesrow")
            warm_rhs = cp.tile([1, JT], F32R, tag="warmrhs")
            warm_sb = cp.tile([1, 4], F32, tag="warm")         # act-table warmup

            # ---- boot: critical DMAs first, then DVE chain in dep order ----
            # x chunk 0 + q/k weights gate the first projection; they go at
            # the head of their queues.  Engine queues are in-order, so the
            # emission order below IS the issue order.
            nc.vector.memset(wqwk_raw[:, :], 0.0)
            nc.sync.dma_start(out=x_raw[:, 0:JT], in_=x_d[:, 0:JT])
            nc.sync.dma_start(out=wqwk_raw[0:D, 0:C], in_=wq_d[:, :])
            nc.sync.dma_start(out=wqwk_raw[0:D, C:2 * C], in_=wk_d[:, :])
            nc.sync.dma_start(out=bias_raw[0:1, 0:D], in_=bq_d[None, :])
            nc.sync.dma_start(out=bias_raw[0:1, D:2 * D], in_=bk_d[None, :])
            nc.scalar.dma_start(out=x_raw[:, JT:2 * JT], in_=x_d[:, JT:2 * JT])
            nc.scalar.dma_start(out=wv_raw[:, :], in_=wv_d[:, :])
            nc.scalar.dma_start(out=bias_raw[0:1, 2 * D:], in_=bv_d[None, :])
            # all x chunks ride HWDGE queues (sync/scalar): SWDGE's multi-us
            # spin-up latency on the gpsimd queue was gating the j-tile-0
            # x-round CASTs, stalling the projection chain
            nc.sync.dma_start(out=x_raw[:, 2 * JT:3 * JT],
                              in_=x_d[:, 2 * JT:3 * JT])
            nc.scalar.dma_start(out=x_raw[:, 3 * JT:4 * JT],
                                in_=x_d[:, 3 * JT:4 * JT])
            nc.sync.dma_start(out=x_raw[:, 4 * JT:5 * JT],
                              in_=x_d[:, 4 * JT:5 * JT])
            nc.scalar.dma_start(out=x_raw[:, 5 * JT:6 * JT],
                                in_=x_d[:, 5 * JT:6 * JT])
            nc.sync.dma_start(out=x_raw[:, 6 * JT:7 * JT],
                              in_=x_d[:, 6 * JT:7 * JT])
            nc.scalar.dma_start(out=x_raw[:, 7 * JT:8 * JT],
                                in_=x_d[:, 7 * JT:8 * JT])
            # constants on GpSimd (keeps the DVE queue free for the boot
            # critical chain); wqk_sb zero covers the unused cols 8-31
            nc.gpsimd.memset(ones_sb[:, :], 1.0)
            nc.gpsimd.memset(vt_sb[:, :, C:C + 1], 1.0)
            nc.gpsimd.memset(ones_rows[:, :].bitcast(F32), 1.0)
            nc.gpsimd.memset(wq_rep[:, :, :].bitcast(F32), 0.0)
            nc.gpsimd.memset(wk_rep[:, :, :].bitcast(F32), 0.0)
            # pre-load the ScalarE activation table during the DMA wait
            nc.scalar.activation(warm_sb[0:1, :],
                                 ones_sb[0:1, 0:1].to_broadcast([1, 4]),
                                 mybir.ActivationFunctionType.Exp)
            # HAM warmup: PE_HAM un-throttles only after a long stretch of
            # busy activity windows; burn the otherwise-idle boot DMA wait on
            # dummy matmuls so the busy counter starts at ~1 us, not ~15 us
            if N_WARM:
                warm_ps = pp.tile([IB, GRP * JT], F32, tag="qkA",
                                  name="warmps")
                for i in range(N_WARM):
                    nc.tensor.matmul(warm_ps[0:C, 0:JT],
                                     lhsT=ones_rows[0:1, :],
                                     rhs=warm_rhs[0:1, :],
                                     start=(i == 0), stop=(i == N_WARM - 1))
            # on-chip transposes: Wq/Wk rows live in wqwk_raw[0:8] (rest
            # zeroed); DVE transposes 32x32 blocks
            for m in range(2):          # 0 = q, 1 = k
                for j in range(2):
                    nc.vector.transpose(
                        wqkT[32 * j:32 * j + 32, 32 * m:32 * m + 32],
                        wqwk_raw[0:32, m * C + 32 * j:m * C + 32 * j + 32])
            # free-dim-broadcast copies place each weight block at the three
            # 32-aligned column positions in one DVE instruction
            nc.vector.tensor_copy(
                wq_rep[0:C, :, 0:D],
                wqkT[0:C, None, 0:D].to_broadcast((C, GRP, D)))
            nc.vector.tensor_copy(
                wq_rep[C:C + 1, :, 0:D],
                bias_raw[0:1, None, 0:D].to_broadcast((1, GRP, D)))
            nc.vector.tensor_copy(
                wk_rep[0:C, :, 0:D],
                wqkT[0:C, None, 32:32 + D].to_broadcast((C, GRP, D)))
            nc.vector.tensor_copy(
                wk_rep[C:C + 1, :, 0:D],
                bias_raw[0:1, None, D:2 * D].to_broadcast((1, GRP, D)))

            x_rounded = [False] * 8
            wv_init = [False]

            def ensure_x(ch):
                """Round x chunk ch (512 wide) to f32r lazily so the startup
                chain doesn't queue behind the whole x preprocessing."""
                cs = slice(ch * JT, (ch + 1) * JT)
                if x_rounded[ch]:
                    return
                x_rounded[ch] = True
                nc.vector.tensor_copy(x_sb[0:C, cs], x_raw[:, cs])
                nc.gpsimd.memset(x_sb[C:C + 1, cs].bitcast(F32), 1.0)

            def emit_proj(ct, w_rep, dst, proj_tile, on_scalar=False):
                """Project one tensor (q or k) for HW-chunk ct (512 wide);
                the replicated stationary lands it at partition groups
                {0, 32, 64} directly, so one [72,512] DVE copy evacuates the
                chunk (garbage rows 8-31/40-63 go along for the ride into
                never-read q_sb/k_sb rows)."""
                ensure_x(ct)
                js = slice(ct * JT, (ct + 1) * JT)
                p = proj_tile()
                nc.tensor.matmul(p[0:64 + D, :],
                                 lhsT=w_rep[:, :, :].rearrange(
                                     "p g c -> p (g c)")[:, 0:64 + D],
                                 rhs=x_sb[:, js], start=True, stop=True)
                if on_scalar:
                    # k chunks c>=1 are first read in j-tile c; their
                    # evacuation rides ScalarE's jt0 slack instead of the
                    # DVE queue (which gates the q/vt critical chains)
                    nc.scalar.activation(dst[:, js], p[0:64 + D, :],
                                         mybir.ActivationFunctionType.Copy)
                else:
                    nc.vector.tensor_copy(dst[:, js], p[0:64 + D, :])

            def emit_vt_proj(vb, proj_tile):
                """Project vT' i-blocks vb*VB .. vb*VB+VB-1."""
                if not wv_init[0]:
                    wv_init[0] = True
                    for i in range(2):
                        for j in range(2):
                            nc.vector.transpose(
                                wvT[32 * j:32 * j + 32, 32 * i:32 * i + 32],
                                wv_raw[32 * i:32 * i + 32, 32 * j:32 * j + 32])
                    nc.vector.tensor_copy(wv_sb[0:C, :], wvT[:, :])
                    nc.vector.tensor_copy(wv_sb[C:C + 1, :],
                                          bias_raw[0:1, 2 * D:])
                ensure_x((vb * VB * IB) // JT)
                ensure_x(((vb + 1) * VB * IB - 1) // JT)
                pv = proj_tile()
                for u in range(VB):
                    ib = vb * VB + u
                    isl = slice(ib * IB, (ib + 1) * IB)
                    nc.tensor.matmul(pv[0:IB, u * C:(u + 1) * C],
                                     lhsT=x_sb[:, isl], rhs=wv_sb[:, :],
                                     start=True, stop=True)
                nc.vector.tensor_copy(
                    vt_sb[:, vb * VB:(vb + 1) * VB, 0:C],
                    pv[:, 0:VB * C].rearrange("p (v c) -> p v c", v=VB))

            def _compute():
                n_grp = (NI + GRP - 1) // GRP
                qk_done = 0
                vt_done = 0
                gidx = [0]         # global group counter (qk ring parity)
                step = [0]
                pend_av = []       # FIFO of (av, att, g, nb, js)
                pend_ep = []       # (av, js)

                def qk_tile():
                    # fp32 logits: TRN2 matmul can only write fp32 PSUM
                    # (16-bit PSUM output is TRN3-only), which pins the exp
                    # ring at 2x3 banks and GRP at 3
                    t = pp.tile([IB, GRP * JT], F32,
                                tag=("qkA" if gidx[0] % 2 == 0 else "qkB"),
                                name="qk")
                    gidx[0] += 1
                    return t

                def av_tile(jt):
                    return pp.tile([IB, JT], F32,
                                   tag=("avA" if jt % 2 == 0 else "avB"),
                                   name="av")

                def proj_tile():
                    # j-tile 0 projection scratch shares avB (av(jt1) is the
                    # next user of that bank, long after the last projection)
                    return pp.tile([IB, JT], F32, tag="avB", name="proj")

                def proj_tileA():
                    # chunk 0's k projection rides the avA bank, which is
                    # free until av(jt0)'s first accumulation at step 2 --
                    # this keeps it off pq(0)'s WAR chain so the first QK
                    # group isn't serialized behind two evacuations
                    return pp.tile([IB, JT], F32, tag="avA", name="projA")

                def ensure_vt(hi_block):
                    nonlocal vt_done
                    while vt_done * VB < hi_block:
                        emit_vt_proj(vt_done, proj_tile)
                        vt_done += 1

                def flush_av():
                    pav, patt, pg, pnb, pjs = pend_av.pop(0)
                    # one-group vt lookahead so the DVE evacuation is queued
                    # well before the AV matmuls that read it
                    ensure_vt(min(NI, pg * GRP + pnb + GRP))
                    for bi in range(pnb):
                        ib = pg * GRP + bi
                        nc.tensor.matmul(
                            pav[0:C + 1, :],
                            lhsT=vt_sb[:, ib, :],
                            rhs=patt[:, bi * JT:(bi + 1) * JT],
                            start=(ib == 0), stop=(ib == NI - 1))
                    if pg * GRP + pnb == NI:
                        pend_ep.append((pav, pjs))

                def flush_ep(final=False):
                    while pend_ep:
                        pav, pjs = pend_ep.pop(0)
                        # reciprocal straight off the PSUM denominator row
                        # (no den evacuation copy); the next j-tile
                        # accumulates into the other av bank, so the PE never
                        # stalls on this epilogue
                        if final:
                            # tail path: the qk ring is idle now, so a ones-
                            # stationary matmul broadcasts the reciprocal into
                            # a ring bank in ~0.5 us (the DMA broadcast below
                            # costs ~6.5 us of descriptor latency, hidden
                            # mid-run but fully exposed at the tail).  The
                            # 3.3 us DVE reciprocal is the tail's long pole,
                            # so run the epilogue in two 256-column halves:
                            # half 1's bcast/mult/add/DMA overlap half 2's
                            # reciprocal.
                            recip_r = wp.tile([IB, JT], F32R, tag="recipr")
                            bcps = pp.tile([IB, JT], F32, tag="qkA",
                                           name="bcps")
                            HJ = JT // 2
                            for h in range(2):
                                hs = slice(h * HJ, (h + 1) * HJ)
                                with nc.allow_low_precision(
                                        reason="f32r round of softmax recip"):
                                    nc.vector.reciprocal(recip_r[64:65, hs],
                                                         pav[64:65, hs])
                                nc.tensor.matmul(bcps[0:C, hs],
                                                 lhsT=ones_rows[64:65, 0:C],
                                                 rhs=recip_r[64:65, hs],
                                                 start=True, stop=True,
                                                 tile_position=(64, 0))
                            for h in range(2):
                                hs = slice(h * HJ, (h + 1) * HJ)
                                hjs = slice(pjs.start + h * HJ,
                                            pjs.start + (h + 1) * HJ)
                                bc_sb = wp.tile([C, HJ], F32, tag="bch")
                                nc.vector.tensor_copy(bc_sb[:, :],
                                                      bcps[0:C, hs])
                                oh = wp.tile([C, HJ], F32, tag="oh")
                                nc.vector.tensor_tensor(
                                    oh[:, :], pav[0:C, hs], bc_sb[:, :],
                                    op=mybir.AluOpType.mult)
                                nc.vector.tensor_tensor(
                                    oh[:, :], oh[:, :], x_raw[:, hjs],
                                    op=mybir.AluOpType.add)
                                nc.sync.dma_start(out=out_d[:, hjs],
                                                  in_=oh[:, :])
                            continue
                        else:
                            recip = wp.tile([IB, JT], F32, tag="recip")
                            nc.vector.reciprocal(recip[64:65, :],
                                                 pav[64:65, :])
                            bc_sb = wp.tile([C, JT], F32, tag="bc")
                            # partition-broadcast DMA: zero-step FREE dim on
                            # the src (the same 2 KB row read 64 times); a
                            # zero-step PARTITION dim is rejected by the DMA
                            # lowering.  ~6.5 us of completion latency, fully
                            # hidden by the next j-tile's compute
                            nc.sync.dma_start(
                                out=bc_sb[:, :],
                                in_=recip[64:65, None, :]
                                .to_broadcast((1, C, JT)))
                            bc_src = bc_sb[:, :]
                        o = wp.tile([C, JT], F32, tag="o")
                        nc.vector.tensor_tensor(o[:, :], pav[0:C, :],
                                                bc_src,
                                                op=mybir.AluOpType.mult)
                        nc.vector.tensor_tensor(o[:, :], o[:, :], x_raw[:, pjs],
                                                op=mybir.AluOpType.add)
                        # SWDGE (gpsimd) issue: that queue is idle mid-run
                        # and nothing waits on the store until the final
                        # drain, so its multi-us latency is free -- while
                        # the sync HWDGE queue stays dedicated to the
                        # broadcast DMA (sharing made the broadcast complete
                        # ~10 us late, thinning av-bank margins)
                        nc.gpsimd.dma_start(out=out_d[:, pjs], in_=o[:, :])

                # prologue: chunk-0 projections precede av(jt0)'s allocation
                # in both bank rings
                emit_proj(0, wq_rep, q_sb, proj_tile)
                emit_proj(0, wk_rep, k_sb, proj_tileA)
                qk_done = 1
                k_done = 1

                for jt in range(NJ):
                    js = slice(jt * JT, (jt + 1) * JT)
                    av = av_tile(jt)
                    for g in range(n_grp):
                        nb = min(GRP, NI - g * GRP)
                        if jt == 0:
                            # round x chunks two groups ahead of the proj
                            # front: the x-round CAST carries a coarse WAR
                            # wait on PE progress (Tile tracks x_sb at tensor
                            # granularity), which stalled the next projection
                            # by ~500-800 ns when emitted just-in-time
                            for ch in range(min(8, g + 3)):
                                ensure_x(ch)
                            # just-in-time q projections, one chunk ahead of
                            # the QK front; k chunk c is only read from
                            # j-tile c on, so the pk projections trickle one
                            # per group (halving early proj-bank WAR stalls)
                            hi_i = (g * GRP + nb) * IB
                            need = min(8, max(1, -(-hi_i // JT)) + 1)
                            while qk_done < need:
                                emit_proj(qk_done, wq_rep, q_sb, proj_tile)
                                qk_done += 1

                            if g == n_grp - 1:
                                while qk_done < 8:
                                    emit_proj(qk_done, wq_rep, q_sb,
                                              proj_tile)
                                    qk_done += 1
                                while k_done < 8:
                                    emit_proj(k_done, wk_rep, k_sb,
                                              proj_tile, on_scalar=True)
                                    k_done += 1
                                # guarantee every avB-bank projection tile is
                                # emitted before av(jt1)'s allocation (the
                                # lazy flush_av path already reaches 32 here)
                                ensure_vt(NI)
                        qk = qk_tile()
                        for bi in range(nb):
                            ib = g * GRP + bi
                            isl = slice(ib * IB, (ib + 1) * IB)
                            nc.tensor.matmul(
                                qk[:, bi * JT:(bi + 1) * JT],
                                lhsT=q_sb[32 * bi:32 * bi + D, isl],
                                rhs=k_sb[32 * bi:32 * bi + D, js],
                                start=True, stop=True,
                                tile_position=(32 * bi, 0))
                        att = wp.tile([IB, GRP * JT], BF16, tag="att")
                        nc.scalar.activation(
                            att[:, 0:nb * JT], qk[:, 0:nb * JT],
                            mybir.ActivationFunctionType.Exp)
                        if jt == 0 and k_done < 8 and g >= 1:
                            # pk lands BETWEEN the QK group and the AV flush
                            # in the PE queue: the QK block covers pq's
                            # evacuation, the AV block covers pk's -- the
                            # single-buffer proj ring then stalls only on
                            # pv groups (ring WAR needs ~830 ns of separying
                            # PE work per projection)
                            emit_proj(k_done, wk_rep, k_sb, proj_tile)
                            k_done += 1
                        flush_ep()
                        pend_av.append((av, att, g, nb, js))
                        while len(pend_av) > AV_LAG:
                            flush_av()
                        step[0] += 1
                while pend_av:
                    flush_av()
                    flush_ep(final=True)
                flush_ep(final=True)

            if loop_n:
                hints = (mybir.EngineType.PE, mybir.EngineType.Activation,
                         mybir.EngineType.DVE, mybir.EngineType.SP,
                         mybir.EngineType.Pool)
                with tc.For_i(0, loop_n, 1, hint_engines=hints):
                    for _ in range(bodies):
                        x_rounded[:] = [False] * 8
                        _compute()
            else:
                _compute()

    _fix_drain_waits(nc)
    return nc


_NC_CACHE = {}


def _get_nc():
    if "nc" not in _NC_CACHE:
        _NC_CACHE["nc"] = build_nc()
    return _NC_CACHE["nc"]


def kernel(**inputs) -> np.ndarray:
    x = np.ascontiguousarray(np.asarray(inputs["x"], dtype=np.float32))
    assert x.shape == (B, C, H, W), x.shape
    weights = {
        name: np.ascontiguousarray(np.asarray(inputs[name], dtype=np.float32))
        for name in ("Wq", "bq", "Wk", "bk", "Wv", "bv")
    }
    in_maps = [{"x": x[b].reshape(C, HW), **weights} for b in range(B)]
    nc = _get_nc()
    res = run_bass_kernel_spmd(nc, in_maps, core_ids=list(range(B)))
    out = np.stack([np.asarray(res.results[b]["out"]).reshape(C, H, W)
                    for b in range(B)])
    return out.astype(np.float32)


# revision 61
# speedup vs baseline: 1.3588x; 1.0308x over previous
"""CrissCross(actually full)-attention Trainium2 kernel.

Reference computation per batch b (C=64 channels, HW=4096 positions, D=8):
    q = Wq@x + bq        [D, HW]
    k = Wk@x + bk        [D, HW]
    v = Wv@x + bv        [C, HW]
    att[i, j] = softmax_i(q[:, i] . k[:, j])
    out[c, j] = sum_i v[c, i] att[i, j] + x[c, j]

Sharding: data-parallel, one batch per NeuronCore (8 cores).

Measured HW model (from NTFF traces of this kernel's runs):
  - QK group (3 row-tiled [8,128]x[8,512] f32r matmuls) is STREAM-bound:
    wall ~727 ns cold (K=4/8, 1.2 GHz) / ~435 ns warm (K=8/8, 2.4 GHz).
  - AV matmul ([128,65]bf16 stationary, 512-col stream): cadence 427 cold /
    241 warm.
  - ScalarE exp of [128, 1536]: 1573 ns cold-phase, 1423 measured in steady
    warm state (ScalarE ~1.33 GHz effective); 88 of them set the ~125-138 us
    ScalarE floor.  Per group: PE cold 2008 ns (PE-paced); PE warm 1158 <
    exp (ScalarE-paced).  Exps are irreducible: 16.7M per core on the only
    exp-capable engine.
  - PE_HAM: the un-throttle grant (K=4/8 -> 8/8) arrives 60-140 us into the
    run at a firmware-paced, effectively random time; early dummy-matmul
    warmup bursts do NOT move it (measured), and grant "blips" die if a PE
    stall lands inside the 3.4 us grant window -- hence the all-out war on
    pipeline gaps below.  Under sustained board heat a separate P0 state
    downclocks PE 2.4->2.0 and ScalarE ~1.33->1.1, inflating everything
    ~15-20%; run-to-run comparisons must account for it.

Per-core dataflow: x'=[x;ones] (biases folded via the ones row); q and k are
projected chunk-by-chunk with REPLICATED stationaries (weight columns at
{0-7,32-39,64-71}) so each 512-wide chunk lands with its row-tiling replicas
in place and one [72,512] DVE copy evacuates it (never-read garbage rows
ride along).  k chunk c is first read in j-tile c, so pk projections trickle
one per group through j-tile 0, emitted BETWEEN the QK group and the AV
flush so each projection's ~830 ns evacuation is covered by a PE work block
(the single-buffer proj ring otherwise stalls the next projection); vT'
(bf16, trailing ones column -> AV emits numerator and denominator together)
is projected 4 blocks per tile with one-group lookahead.  x chunks are
rounded two groups ahead of the projection front.  Softmax skips
max-subtraction (|logit| < ~26, well inside fp32/bf16 exp range).
NOTE: pk deferral into later j-tiles' "idle" av bank was measured at
-4..6 us PER J-TILE: the mid-run epilogue's broadcast DMA completes ~10 us
after the boundary, so neither av bank is actually free mid-j-tile.

PSUM layout (single pool, 8 banks, explicit tags):
  qkA [128,1536] banks 0-2   |  exp-input ring, alternating per group
  qkB [128,1536] banks 3-5   |  (global group parity across j-tiles)
  avA [128, 512] bank 6      |  AV accumulator, alternating per j-tile
  avB [128, 512] bank 7      |  parity; j-tile 0's projection scratch
                                also rotates through these banks
Epilogue per j-tile (runs while the next j-tile computes, PE-free):
  DVE reciprocal reads the denominator row (av[64]) straight from PSUM; a
  partition-broadcast DMA (zero-step FREE dim; ~6.5 us descriptor latency,
  fully hidden) replicates it to 64 rows; DVE mult (+x residual) and DMA
  out.  The next j-tile accumulates into the other av bank, so the PE never
  stalls on the epilogue -- stall-free j-tile boundaries are what let HAM
  warm windows survive once granted.  The LAST j-tile instead broadcasts
  via a ones-stationary matmul into the now-idle qk ring (tile_position
  (64,0)) in two 256-column halves, hiding half the 3.3 us reciprocal.
"""

import numpy as np

import bass_rust
import concourse.bass as bass
import concourse.tile as tile
from concourse import mybir
from concourse.bass_utils import run_bass_kernel_spmd

B, C, HW, D = 8, 64, 4096, 8
H = W = 64
JT = 512          # j-tile width (PSUM bank)
NJ = HW // JT     # 8
IB = 128          # i-block height (partitions)
NI = HW // IB     # 32
GRP = 3           # i-blocks per exp group (3-way row tiling)
N_WARM = 0        # boot HAM-warmup dummy matmuls: measured useless (the
                  # PE_HAM un-throttle grant is firmware-paced, ~85-127 us
                  # into the run regardless of early PE activity)
VB = 4            # vT i-blocks projected per PSUM tile
AV_LAG = 2        # groups the AV flush trails the QK/exp front
TAIL_LAG = 2      # groups the PE bcast trails the epilogue's reciprocal

F32 = mybir.dt.float32
F32R = mybir.dt.float32r
F16 = mybir.dt.float16
BF16 = mybir.dt.bfloat16


def _fix_drain_waits(nc):
    """walrus in this container rejects instructions carrying more than one
    sync-wait; hoist extras onto NoOps inserted just before, same engine."""
    for f in nc.m.functions:
        for blk in f.blocks:
            insts = blk.instructions
            for tgt in [
                i for i in list(insts)
                if i.sync_info and len(i.sync_info.on_wait or []) > 1
            ]:
                si = tgt.sync_info
                waits = list(si.on_wait)
                si.on_wait = waits[-1:]
                di = insts.index(tgt)
                for w in waits[:-1]:
                    n = nc.engines[tgt.engine].nop()
                    for b in f.blocks:
                        bi = b.instructions
                        for idx in range(len(bi) - 1, -1, -1):
                            if bi[idx].name == n.ins.name:
                                bi.pop(idx)
                                break
                    n.ins.sync_info = bass_rust.SyncInfo(on_wait=[w], on_update=[])
                    insts.insert(di, n.ins)
                    di += 1


def build_nc(loop_n=None, bodies=1):
    nc = bass.Bass()
    x_d = nc.dram_tensor("x", [C, HW], F32, kind="ExternalInput")
    wq_d = nc.dram_tensor("Wq", [D, C], F32, kind="ExternalInput")
    bq_d = nc.dram_tensor("bq", [D], F32, kind="ExternalInput")
    wk_d = nc.dram_tensor("Wk", [D, C], F32, kind="ExternalInput")
    bk_d = nc.dram_tensor("bk", [D], F32, kind="ExternalInput")
    wv_d = nc.dram_tensor("Wv", [C, C], F32, kind="ExternalInput")
    bv_d = nc.dram_tensor("bv", [C], F32, kind="ExternalInput")
    out_d = nc.dram_tensor("out", [C, HW], F32, kind="ExternalOutput")

    with tile.TileContext(nc) as tc:
        with (
            tc.tile_pool(name="const", bufs=1) as cp,
            tc.tile_pool(name="work", bufs=4) as wp,
            tc.tile_pool(name="qtmp", bufs=2) as qp,
            tc.tile_pool(name="ps", bufs=1, space="PSUM") as pp,
        ):
            # ---- persistent SBUF tensors ----
            x_raw = cp.tile([C, HW], F32, tag="xraw")        # residual source
            x_sb = cp.tile([C + 1, HW], F32R, tag="x")       # x' = [x; ones]
            # raw weights land CONTIGUOUS and are transposed on-chip (DVE
            # 32x32 block transposes); transposing DMAs cost us of descriptors
            wqwk_raw = cp.tile([32, 2 * C], F32, tag="wqwkraw")
            wv_raw = cp.tile([C, C], F32, tag="wvraw")
            bias_raw = cp.tile([1, 2 * D + C], F32, tag="braw")
            wqkT = cp.tile([C, C], F32, tag="wqkT")            # [WqT | WkT]
            wvT = cp.tile([C, C], F32, tag="wvT")
            # replicated projection stationaries: weight columns at
            # {0-7, 32-39, 64-71} so the projection matmul emits q (resp. k)
            # with its row-tiling replicas already in place -- one [72,512]
            # DVE copy evacuates a whole chunk (cols 8-31/40-63 are never
            # read downstream, so they stay uninitialized)
            wqk_rep = cp.tile([C + 1, 4, 32], F32R, tag="wqkrep")
            wv_sb = cp.tile([C + 1, C], F32R, tag="wv")        # [WvT; bv]
            q_sb = cp.tile([96 + D, HW], F32R, tag="q")  # q @0/32/64, k @96
            k_sb = cp.tile([64 + D, HW], F32R, tag="k")
            vt_sb = cp.tile([IB, NI, C + 1], BF16, tag="vt")   # vT' blocks
            ones_sb = cp.tile([IB, 1], F32, tag="ones")
            # tail-epilogue bcast stationary (row 64) + HAM-warmup operands
            ones_rows = cp.tile([IB, C], F32R, tag="onesrow")
            warm_rhs = cp.tile([1, JT], F32R, tag="warmrhs")
            warm_sb = cp.tile([1, 4], F32, tag="warm")         # act-table warmup

            # ---- boot: critical DMAs first, then DVE chain in dep order ----
            # x chunk 0 + q/k weights gate the first projection; they go at
            # the head of their queues.  Engine queues are in-order, so the
            # emission order below IS the issue order.
            nc.vector.memset(wqwk_raw[:, :], 0.0)
            nc.sync.dma_start(out=x_raw[:, 0:JT], in_=x_d[:, 0:JT])
            nc.sync.dma_start(out=wqwk_raw[0:D, 0:C], in_=wq_d[:, :])
            nc.sync.dma_start(out=wqwk_raw[0:D, C:2 * C], in_=wk_d[:, :])
            nc.sync.dma_start(out=bias_raw[0:1, 0:D], in_=bq_d[None, :])
            nc.sync.dma_start(out=bias_raw[0:1, D:2 * D], in_=bk_d[None, :])
            nc.scalar.dma_start(out=x_raw[:, JT:2 * JT], in_=x_d[:, JT:2 * JT])
            nc.scalar.dma_start(out=wv_raw[:, :], in_=wv_d[:, :])
            nc.scalar.dma_start(out=bias_raw[0:1, 2 * D:], in_=bv_d[None, :])
            # all x chunks ride HWDGE queues (sync/scalar): SWDGE's multi-us
            # spin-up latency on the gpsimd queue was gating the j-tile-0
            # x-round CASTs, stalling the projection chain
            nc.sync.dma_start(out=x_raw[:, 2 * JT:3 * JT],
                              in_=x_d[:, 2 * JT:3 * JT])
            nc.scalar.dma_start(out=x_raw[:, 3 * JT:4 * JT],
                                in_=x_d[:, 3 * JT:4 * JT])
            nc.sync.dma_start(out=x_raw[:, 4 * JT:5 * JT],
                              in_=x_d[:, 4 * JT:5 * JT])
            nc.scalar.dma_start(out=x_raw[:, 5 * JT:6 * JT],
                                in_=x_d[:, 5 * JT:6 * JT])
            nc.sync.dma_start(out=x_raw[:, 6 * JT:7 * JT],
                              in_=x_d[:, 6 * JT:7 * JT])
            nc.scalar.dma_start(out=x_raw[:, 7 * JT:8 * JT],
                                in_=x_d[:, 7 * JT:8 * JT])
            # constants on GpSimd (keeps the DVE queue free for the boot
            # critical chain); wqk_sb zero covers the unused cols 8-31
            nc.gpsimd.memset(ones_sb[:, :], 1.0)
            nc.gpsimd.memset(vt_sb[:, :, C:C + 1], 1.0)
            nc.gpsimd.memset(ones_rows[:, :].bitcast(F32), 1.0)
            nc.gpsimd.memset(wqk_rep[:, :, :].bitcast(F32), 0.0)
            # pre-load the ScalarE activation table during the DMA wait
            nc.scalar.activation(warm_sb[0:1, :],
                                 ones_sb[0:1, 0:1].to_broadcast([1, 4]),
                                 mybir.ActivationFunctionType.Exp)
            # HAM warmup: PE_HAM un-throttles only after a long stretch of
            # busy activity windows; burn the otherwise-idle boot DMA wait on
            # dummy matmuls so the busy counter starts at ~1 us, not ~15 us
            if N_WARM:
                warm_ps = pp.tile([IB, GRP * JT], F32, tag="qkA",
                                  name="warmps")
                for i in range(N_WARM):
                    nc.tensor.matmul(warm_ps[0:C, 0:JT],
                                     lhsT=ones_rows[0:1, :],
                                     rhs=warm_rhs[0:1, :],
                                     start=(i == 0), stop=(i == N_WARM - 1))
            # on-chip transposes: Wq/Wk rows live in wqwk_raw[0:8] (rest
            # zeroed); DVE transposes 32x32 blocks
            for m in range(2):          # 0 = q, 1 = k
                for j in range(2):
                    nc.vector.transpose(
                        wqkT[32 * j:32 * j + 32, 32 * m:32 * m + 32],
                        wqwk_raw[0:32, m * C + 32 * j:m * C + 32 * j + 32])
            # free-dim-broadcast copies place each weight block at the three
            # 32-aligned column positions in one DVE instruction
            nc.vector.tensor_copy(
                wqk_rep[0:C, 0:GRP, 0:D],
                wqkT[0:C, None, 0:D].to_broadcast((C, GRP, D)))
            nc.vector.tensor_copy(
                wqk_rep[C:C + 1, 0:GRP, 0:D],
                bias_raw[0:1, None, 0:D].to_broadcast((1, GRP, D)))
            nc.vector.tensor_copy(wqk_rep[0:C, 3, 0:D],
                                  wqkT[0:C, 32:32 + D])
            nc.vector.tensor_copy(wqk_rep[C:C + 1, 3, 0:D],
                                  bias_raw[0:1, D:2 * D])

            x_rounded = [False] * 8
            wv_init = [False]

            def ensure_x(ch):
                """Round x chunk ch (512 wide) to f32r lazily so the startup
                chain doesn't queue behind the whole x preprocessing."""
                cs = slice(ch * JT, (ch + 1) * JT)
                if x_rounded[ch]:
                    return
                x_rounded[ch] = True
                nc.vector.tensor_copy(x_sb[0:C, cs], x_raw[:, cs])
                nc.gpsimd.memset(x_sb[C:C + 1, cs].bitcast(F32), 1.0)

            def emit_proj(ct, proj_tile):
                """Project q AND k for HW-chunk ct (512 wide) in ONE
                matmul: the replicated stationary lands q at partition
                groups {0, 32, 64} and k once at {96-103}; a single
                [104,512] DVE copy evacuates everything (garbage rows ride
                into never-read q_sb rows).  k is then scattered to its
                {0, 32, 64} positions by SWDGE DMAs (chunk c is first read
                in j-tile c, so the multi-us SWDGE latency is free)."""
                ensure_x(ct)
                js = slice(ct * JT, (ct + 1) * JT)
                p = proj_tile()
                nc.tensor.matmul(p[0:96 + D, :],
                                 lhsT=wqk_rep[:, :, :].rearrange(
                                     "p g c -> p (g c)")[:, 0:96 + D],
                                 rhs=x_sb[:, js], start=True, stop=True)
                nc.vector.tensor_copy(q_sb[:, js], p[0:96 + D, :])

            def scatter_k(ct, on_dve=False):
                js = slice(ct * JT, (ct + 1) * JT)
                for r in (0, 32, 64):
                    if on_dve:
                        nc.vector.tensor_copy(k_sb[r:r + D, js],
                                              q_sb[96:96 + D, js])
                    else:
                        nc.gpsimd.dma_start(out=k_sb[r:r + D, js],
                                            in_=q_sb[96:96 + D, js])

            def emit_vt_proj(vb, proj_tile):
                """Project vT' i-blocks vb*VB .. vb*VB+VB-1."""
                if not wv_init[0]:
                    wv_init[0] = True
                    for i in range(2):
                        for j in range(2):
                            nc.vector.transpose(
                                wvT[32 * j:32 * j + 32, 32 * i:32 * i + 32],
                                wv_raw[32 * i:32 * i + 32, 32 * j:32 * j + 32])
                    nc.vector.tensor_copy(wv_sb[0:C, :], wvT[:, :])
                    nc.vector.tensor_copy(wv_sb[C:C + 1, :],
                                          bias_raw[0:1, 2 * D:])
                ensure_x((vb * VB * IB) // JT)
                ensure_x(((vb + 1) * VB * IB - 1) // JT)
                pv = proj_tile()
                for u in range(VB):
                    ib = vb * VB + u
                    isl = slice(ib * IB, (ib + 1) * IB)
                    nc.tensor.matmul(pv[0:IB, u * C:(u + 1) * C],
                                     lhsT=x_sb[:, isl], rhs=wv_sb[:, :],
                                     start=True, stop=True)
                nc.vector.tensor_copy(
                    vt_sb[:, vb * VB:(vb + 1) * VB, 0:C],
                    pv[:, 0:VB * C].rearrange("p (v c) -> p v c", v=VB))

            def _compute():
                n_grp = (NI + GRP - 1) // GRP
                qk_done = 0
                vt_done = 0
                gidx = [0]         # global group counter (qk ring parity)
                step = [0]
                pend_av = []       # FIFO of (av, att, g, nb, js)
                pend_ep = []       # (av, js)

                def qk_tile():
                    # fp32 logits: TRN2 matmul can only write fp32 PSUM
                    # (16-bit PSUM output is TRN3-only), which pins the exp
                    # ring at 2x3 banks and GRP at 3
                    t = pp.tile([IB, GRP * JT], F32,
                                tag=("qkA" if gidx[0] % 2 == 0 else "qkB"),
                                name="qk")
                    gidx[0] += 1
                    return t

                def av_tile(jt):
                    return pp.tile([IB, JT], F32,
                                   tag=("avA" if jt % 2 == 0 else "avB"),
                                   name="av")

                def proj_tile():
                    # j-tile 0 projection scratch shares avB (av(jt1) is the
                    # next user of that bank, long after the last projection)
                    return pp.tile([IB, JT], F32, tag="avB", name="proj")

                def proj_tileA():
                    # chunk 0's k projection rides the avA bank, which is
                    # free until av(jt0)'s first accumulation at step 2 --
                    # this keeps it off pq(0)'s WAR chain so the first QK
                    # group isn't serialized behind two evacuations
                    return pp.tile([IB, JT], F32, tag="avA", name="projA")

                def ensure_vt(hi_block):
                    nonlocal vt_done
                    while vt_done * VB < hi_block:
                        emit_vt_proj(vt_done, proj_tile)
                        vt_done += 1

                def flush_av():
                    pav, patt, pg, pnb, pjs = pend_av.pop(0)
                    # one-group vt lookahead so the DVE evacuation is queued
                    # well before the AV matmuls that read it
                    ensure_vt(min(NI, pg * GRP + pnb + GRP))
                    for bi in range(pnb):
                        ib = pg * GRP + bi
                        nc.tensor.matmul(
                            pav[0:C + 1, :],
                            lhsT=vt_sb[:, ib, :],
                            rhs=patt[:, bi * JT:(bi + 1) * JT],
                            start=(ib == 0), stop=(ib == NI - 1))
                    if pg * GRP + pnb == NI:
                        pend_ep.append((pav, pjs))

                def flush_ep(final=False):
                    while pend_ep:
                        pav, pjs = pend_ep.pop(0)
                        # reciprocal straight off the PSUM denominator row
                        # (no den evacuation copy); the next j-tile
                        # accumulates into the other av bank, so the PE never
                        # stalls on this epilogue
                        if final:
                            # tail path: the qk ring is idle now, so a ones-
                            # stationary matmul broadcasts the reciprocal into
                            # a ring bank in ~0.5 us (the DMA broadcast below
                            # costs ~6.5 us of descriptor latency, hidden
                            # mid-run but fully exposed at the tail).  The
                            # 3.3 us DVE reciprocal is the tail's long pole,
                            # so run the epilogue in two 256-column halves:
                            # half 1's bcast/mult/add/DMA overlap half 2's
                            # reciprocal.
                            recip_r = wp.tile([IB, JT], F32R, tag="recipr")
                            bcps = pp.tile([IB, JT], F32, tag="qkA",
                                           name="bcps")
                            HJ = JT // 2
                            for h in range(2):
                                hs = slice(h * HJ, (h + 1) * HJ)
                                with nc.allow_low_precision(
                                        reason="f32r round of softmax recip"):
                                    nc.vector.reciprocal(recip_r[64:65, hs],
                                                         pav[64:65, hs])
                                nc.tensor.matmul(bcps[0:C, hs],
                                                 lhsT=ones_rows[64:65, 0:C],
                                                 rhs=recip_r[64:65, hs],
                                                 start=True, stop=True,
                                                 tile_position=(64, 0))
                            for h in range(2):
                                hs = slice(h * HJ, (h + 1) * HJ)
                                hjs = slice(pjs.start + h * HJ,
                                            pjs.start + (h + 1) * HJ)
                                bc_sb = wp.tile([C, HJ], F32, tag="bch")
                                nc.vector.tensor_copy(bc_sb[:, :],
                                                      bcps[0:C, hs])
                                oh = wp.tile([C, HJ], F32, tag="oh")
                                nc.vector.tensor_tensor(
                                    oh[:, :], pav[0:C, hs], bc_sb[:, :],
                                    op=mybir.AluOpType.mult)
                                nc.vector.tensor_tensor(
                                    oh[:, :], oh[:, :], x_raw[:, hjs],
                                    op=mybir.AluOpType.add)
                                nc.sync.dma_start(out=out_d[:, hjs],
                                                  in_=oh[:, :])
                            continue
                        else:
                            recip = wp.tile([IB, JT], F32, tag="recip")
                            nc.vector.reciprocal(recip[64:65, :],
                                                 pav[64:65, :])
                            bc_sb = wp.tile([C, JT], F32, tag="bc")
                            # partition-broadcast DMA: zero-step FREE dim on
                            # the src (the same 2 KB row read 64 times); a
                            # zero-step PARTITION dim is rejected by the DMA
                            # lowering.  ~6.5 us of completion latency, fully
                            # hidden by the next j-tile's compute
                            nc.sync.dma_start(
                                out=bc_sb[:, :],
                                in_=recip[64:65, None, :]
                                .to_broadcast((1, C, JT)))
                            bc_src = bc_sb[:, :]
                        o = wp.tile([C, JT], F32, tag="o")
                        nc.vector.tensor_tensor(o[:, :], pav[0:C, :],
                                                bc_src,
                                                op=mybir.AluOpType.mult)
                        nc.vector.tensor_tensor(o[:, :], o[:, :], x_raw[:, pjs],
                                                op=mybir.AluOpType.add)
                        # SWDGE (gpsimd) issue: that queue is idle mid-run
                        # and nothing waits on the store until the final
                        # drain, so its multi-us latency is free -- while
                        # the sync HWDGE queue stays dedicated to the
                        # broadcast DMA (sharing made the broadcast complete
                        # ~10 us late, thinning av-bank margins)
                        nc.gpsimd.dma_start(out=out_d[:, pjs], in_=o[:, :])

                # prologue: the fused chunk-0 projection precedes av(jt0)'s
                # allocation in the proj ring; its k scatter rides DVE (the
                # first QK group needs it right away)
                emit_proj(0, proj_tile)
                scatter_k(0, on_dve=True)
                qk_done = 1
                k_done = 1

                for jt in range(NJ):
                    js = slice(jt * JT, (jt + 1) * JT)
                    av = av_tile(jt)
                    for g in range(n_grp):
                        nb = min(GRP, NI - g * GRP)
                        if jt == 0:
                            # round x chunks two groups ahead of the proj
                            # front: the x-round CAST carries a coarse WAR
                            # wait on PE progress (Tile tracks x_sb at tensor
                            # granularity), which stalled the next projection
                            # by ~500-800 ns when emitted just-in-time
                            for ch in range(min(8, g + 3)):
                                ensure_x(ch)
                            # just-in-time q projections, one chunk ahead of
                            # the QK front; k chunk c is only read from
                            # j-tile c on, so the pk projections trickle one
                            # per group (halving early proj-bank WAR stalls)
                            hi_i = (g * GRP + nb) * IB
                            need = min(8, max(1, -(-hi_i // JT)) + 1)
                            while qk_done < need:
                                emit_proj(qk_done, proj_tile)
                                qk_done += 1

                            if g == n_grp - 1:
                                while qk_done < 8:
                                    emit_proj(qk_done, proj_tile)
                                    qk_done += 1
                                while k_done < 8:
                                    scatter_k(k_done)
                                    k_done += 1
                                # guarantee every avB-bank projection tile is
                                # emitted before av(jt1)'s allocation (the
                                # lazy flush_av path already reaches 32 here)
                                ensure_vt(NI)
                        qk = qk_tile()
                        for bi in range(nb):
                            ib = g * GRP + bi
                            isl = slice(ib * IB, (ib + 1) * IB)
                            nc.tensor.matmul(
                                qk[:, bi * JT:(bi + 1) * JT],
                                lhsT=q_sb[32 * bi:32 * bi + D, isl],
                                rhs=k_sb[32 * bi:32 * bi + D, js],
                                start=True, stop=True,
                                tile_position=(32 * bi, 0))
                        att = wp.tile([IB, GRP * JT], BF16, tag="att")
                        nc.scalar.activation(
                            att[:, 0:nb * JT], qk[:, 0:nb * JT],
                            mybir.ActivationFunctionType.Exp)
                        if jt == 0 and k_done < min(8, qk_done) and g >= 1:
                            # k chunk k_done's staging rows exist once its
                            # fused projection ran; scatter via SWDGE
                            scatter_k(k_done)
                            k_done += 1
                        flush_ep()
                        pend_av.append((av, att, g, nb, js))
                        while len(pend_av) > AV_LAG:
                            flush_av()
                        step[0] += 1
                while pend_av:
                    flush_av()
                    flush_ep(final=True)
                flush_ep(final=True)

            if loop_n:
                hints = (mybir.EngineType.PE, mybir.EngineType.Activation,
                         mybir.EngineType.DVE, mybir.EngineType.SP,
                         mybir.EngineType.Pool)
                with tc.For_i(0, loop_n, 1, hint_engines=hints):
                    for _ in range(bodies):
                        x_rounded[:] = [False] * 8
                        _compute()
            else:
                _compute()

    _fix_drain_waits(nc)
    return nc


_NC_CACHE = {}


def _get_nc():
    if "nc" not in _NC_CACHE:
        _NC_CACHE["nc"] = build_nc()
    return _NC_CACHE["nc"]


def kernel(**inputs) -> np.ndarray:
    x = np.ascontiguousarray(np.asarray(inputs["x"], dtype=np.float32))
    assert x.shape == (B, C, H, W), x.shape
    weights = {
        name: np.ascontiguousarray(np.asarray(inputs[name], dtype=np.float32))
        for name in ("Wq", "bq", "Wk", "bk", "Wv", "bv")
    }
    in_maps = [{"x": x[b].reshape(C, HW), **weights} for b in range(B)]
    nc = _get_nc()
    res = run_bass_kernel_spmd(nc, in_maps, core_ids=list(range(B)))
    out = np.stack([np.asarray(res.results[b]["out"]).reshape(C, H, W)
                    for b in range(B)])
    return out.astype(np.float32)


# revision 62
# speedup vs baseline: 1.3788x; 1.0147x over previous
"""CrissCross(actually full)-attention Trainium2 kernel.

Reference computation per batch b (C=64 channels, HW=4096 positions, D=8):
    q = Wq@x + bq        [D, HW]
    k = Wk@x + bk        [D, HW]
    v = Wv@x + bv        [C, HW]
    att[i, j] = softmax_i(q[:, i] . k[:, j])
    out[c, j] = sum_i v[c, i] att[i, j] + x[c, j]

Sharding: data-parallel, one batch per NeuronCore (8 cores).

Measured HW model (from NTFF traces of this kernel's runs):
  - QK group (3 row-tiled [8,128]x[8,512] f32r matmuls) is STREAM-bound:
    wall ~727 ns cold (K=4/8, 1.2 GHz) / ~435 ns warm (K=8/8, 2.4 GHz).
  - AV matmul ([128,65]bf16 stationary, 512-col stream): cadence 427 cold /
    241 warm.
  - ScalarE exp of [128, 1536]: 1573 ns cold-phase, 1423 measured in steady
    warm state (ScalarE ~1.33 GHz effective); 88 of them set the ~125-138 us
    ScalarE floor.  Per group: PE cold 2008 ns (PE-paced); PE warm 1158 <
    exp (ScalarE-paced).  Exps are irreducible: 16.7M per core on the only
    exp-capable engine.
  - PE_HAM: the un-throttle grant (K=4/8 -> 8/8) arrives 60-140 us into the
    run at a firmware-paced, effectively random time; early dummy-matmul
    warmup bursts do NOT move it (measured), and grant "blips" die if a PE
    stall lands inside the 3.4 us grant window -- hence the all-out war on
    pipeline gaps below.  Under sustained board heat a separate P0 state
    downclocks PE 2.4->2.0 and ScalarE ~1.33->1.1, inflating everything
    ~15-20%; run-to-run comparisons must account for it.

Per-core dataflow: x'=[x;ones] (biases folded via the ones row); q AND k are
projected chunk-by-chunk in ONE fused matmul per 512-wide chunk: the
replicated stationary puts q's weight columns at {0-7,32-39,64-71} and k's
at {96-103}, so the output lands with q's row-tiling replicas in place and
k staged in the free partition band; a single [104,512] DVE copy evacuates
everything (never-read garbage rows ride along), and k is scattered to its
{0,32,64} rhs positions by SWDGE DMAs (k chunk c is first read in j-tile c,
so the multi-us SWDGE latency is free; chunk 0 scatters on DVE).  This
halves the projection matmul count and proj-ring traffic -- j-tile 0 runs
stall-free.  vT' (bf16, trailing ones column -> AV emits numerator and
denominator together) is projected 4 blocks per tile with one-group
lookahead; x chunks are rounded two groups ahead.  Softmax skips
max-subtraction (|logit| < ~26, well inside fp32/bf16 exp range).
NOTE: pk deferral into later j-tiles' "idle" av bank was measured at
-4..6 us PER J-TILE: the mid-run epilogue's broadcast DMA completes ~10 us
after the boundary, so neither av bank is actually free mid-j-tile.

PSUM layout (single pool, 8 banks, explicit tags):
  qkA [128,1536] banks 0-2   |  exp-input ring, alternating per group
  qkB [128,1536] banks 3-5   |  (global group parity across j-tiles)
  avA [128, 512] bank 6      |  AV accumulator, alternating per j-tile
  avB [128, 512] bank 7      |  parity; j-tile 0's projection scratch
                                also rotates through these banks
Epilogue per j-tile (runs while the next j-tile computes, PE-free):
  DVE reciprocal reads the denominator row (av[64]) straight from PSUM; a
  partition-broadcast DMA (zero-step FREE dim; ~6.5 us descriptor latency,
  fully hidden) replicates it to 64 rows; DVE mult (+x residual) and DMA
  out.  The next j-tile accumulates into the other av bank, so the PE never
  stalls on the epilogue -- stall-free j-tile boundaries are what let HAM
  warm windows survive once granted.  The LAST j-tile instead broadcasts
  via a ones-stationary matmul into the now-idle qk ring (tile_position
  (64,0)) in two 256-column halves, hiding half the 3.3 us reciprocal.
"""

import numpy as np

import bass_rust
import concourse.bass as bass
import concourse.tile as tile
from concourse import mybir
from concourse.bass_utils import run_bass_kernel_spmd

B, C, HW, D = 8, 64, 4096, 8
H = W = 64
JT = 512          # j-tile width (PSUM bank)
NJ = HW // JT     # 8
IB = 128          # i-block height (partitions)
NI = HW // IB     # 32
GRP = 3           # i-blocks per exp group (3-way row tiling)
N_WARM = 0        # boot HAM-warmup dummy matmuls: measured useless (the
                  # PE_HAM un-throttle grant is firmware-paced, ~85-127 us
                  # into the run regardless of early PE activity)
VB = 4            # vT i-blocks projected per PSUM tile
AV_LAG = 2        # groups the AV flush trails the QK/exp front
TAIL_LAG = 2      # groups the PE bcast trails the epilogue's reciprocal

F32 = mybir.dt.float32
F32R = mybir.dt.float32r
F16 = mybir.dt.float16
BF16 = mybir.dt.bfloat16


def _fix_drain_waits(nc):
    """walrus in this container rejects instructions carrying more than one
    sync-wait; hoist extras onto NoOps inserted just before, same engine."""
    for f in nc.m.functions:
        for blk in f.blocks:
            insts = blk.instructions
            for tgt in [
                i for i in list(insts)
                if i.sync_info and len(i.sync_info.on_wait or []) > 1
            ]:
                si = tgt.sync_info
                waits = list(si.on_wait)
                si.on_wait = waits[-1:]
                di = insts.index(tgt)
                for w in waits[:-1]:
                    n = nc.engines[tgt.engine].nop()
                    for b in f.blocks:
                        bi = b.instructions
                        for idx in range(len(bi) - 1, -1, -1):
                            if bi[idx].name == n.ins.name:
                                bi.pop(idx)
                                break
                    n.ins.sync_info = bass_rust.SyncInfo(on_wait=[w], on_update=[])
                    insts.insert(di, n.ins)
                    di += 1


def build_nc(loop_n=None, bodies=1):
    nc = bass.Bass()
    x_d = nc.dram_tensor("x", [C, HW], F32, kind="ExternalInput")
    wq_d = nc.dram_tensor("Wq", [D, C], F32, kind="ExternalInput")
    bq_d = nc.dram_tensor("bq", [D], F32, kind="ExternalInput")
    wk_d = nc.dram_tensor("Wk", [D, C], F32, kind="ExternalInput")
    bk_d = nc.dram_tensor("bk", [D], F32, kind="ExternalInput")
    wv_d = nc.dram_tensor("Wv", [C, C], F32, kind="ExternalInput")
    bv_d = nc.dram_tensor("bv", [C], F32, kind="ExternalInput")
    out_d = nc.dram_tensor("out", [C, HW], F32, kind="ExternalOutput")

    with tile.TileContext(nc) as tc:
        with (
            tc.tile_pool(name="const", bufs=1) as cp,
            tc.tile_pool(name="work", bufs=4) as wp,
            tc.tile_pool(name="qtmp", bufs=2) as qp,
            tc.tile_pool(name="ps", bufs=1, space="PSUM") as pp,
        ):
            # ---- persistent SBUF tensors ----
            x_raw = cp.tile([C, HW], F32, tag="xraw")        # residual source
            x_sb = cp.tile([C + 1, HW], F32R, tag="x")       # x' = [x; ones]
            # raw weights land CONTIGUOUS and are transposed on-chip (DVE
            # 32x32 block transposes); transposing DMAs cost us of descriptors
            wqwk_raw = cp.tile([32, 2 * C], F32, tag="wqwkraw")
            wv_raw = cp.tile([C, C], F32, tag="wvraw")
            bias_raw = cp.tile([1, 2 * D + C], F32, tag="braw")
            wqkT = cp.tile([C, C], F32, tag="wqkT")            # [WqT | WkT]
            wvT = cp.tile([C, C], F32, tag="wvT")
            # replicated projection stationaries: weight columns at
            # {0-7, 32-39, 64-71} so the projection matmul emits q (resp. k)
            # with its row-tiling replicas already in place -- one [72,512]
            # DVE copy evacuates a whole chunk (cols 8-31/40-63 are never
            # read downstream, so they stay uninitialized)
            wqk_rep = cp.tile([C + 1, 4, 32], F32R, tag="wqkrep")
            wv_sb = cp.tile([C + 1, C], F32R, tag="wv")        # [WvT; bv]
            q_sb = cp.tile([96 + D, HW], F32R, tag="q")  # q @0/32/64, k @96
            k_sb = cp.tile([64 + D, HW], F32R, tag="k")
            vt_sb = cp.tile([IB, NI, C + 1], BF16, tag="vt")   # vT' blocks
            ones_sb = cp.tile([IB, 1], F32, tag="ones")
            # tail-epilogue bcast stationary (row 64) + HAM-warmup operands
            ones_rows = cp.tile([IB, C], F32R, tag="onesrow")
            warm_rhs = cp.tile([1, JT], F32R, tag="warmrhs")
            warm_sb = cp.tile([1, 4], F32, tag="warm")         # act-table warmup

            # ---- boot: critical DMAs first, then DVE chain in dep order ----
            # x chunk 0 + q/k weights gate the first projection; they go at
            # the head of their queues.  Engine queues are in-order, so the
            # emission order below IS the issue order.
            nc.vector.memset(wqwk_raw[:, :], 0.0)
            nc.sync.dma_start(out=x_raw[:, 0:JT], in_=x_d[:, 0:JT])
            nc.sync.dma_start(out=wqwk_raw[0:D, 0:C], in_=wq_d[:, :])
            nc.sync.dma_start(out=wqwk_raw[0:D, C:2 * C], in_=wk_d[:, :])
            nc.sync.dma_start(out=bias_raw[0:1, 0:D], in_=bq_d[None, :])
            nc.sync.dma_start(out=bias_raw[0:1, D:2 * D], in_=bk_d[None, :])
            nc.scalar.dma_start(out=x_raw[:, JT:2 * JT], in_=x_d[:, JT:2 * JT])
            nc.scalar.dma_start(out=wv_raw[:, :], in_=wv_d[:, :])
            nc.scalar.dma_start(out=bias_raw[0:1, 2 * D:], in_=bv_d[None, :])
            # all x chunks ride HWDGE queues (sync/scalar): SWDGE's multi-us
            # spin-up latency on the gpsimd queue was gating the j-tile-0
            # x-round CASTs, stalling the projection chain
            nc.sync.dma_start(out=x_raw[:, 2 * JT:3 * JT],
                              in_=x_d[:, 2 * JT:3 * JT])
            nc.scalar.dma_start(out=x_raw[:, 3 * JT:4 * JT],
                                in_=x_d[:, 3 * JT:4 * JT])
            nc.sync.dma_start(out=x_raw[:, 4 * JT:5 * JT],
                              in_=x_d[:, 4 * JT:5 * JT])
            nc.scalar.dma_start(out=x_raw[:, 5 * JT:6 * JT],
                                in_=x_d[:, 5 * JT:6 * JT])
            nc.sync.dma_start(out=x_raw[:, 6 * JT:7 * JT],
                              in_=x_d[:, 6 * JT:7 * JT])
            nc.scalar.dma_start(out=x_raw[:, 7 * JT:8 * JT],
                                in_=x_d[:, 7 * JT:8 * JT])
            # constants on GpSimd (keeps the DVE queue free for the boot
            # critical chain); wqk_sb zero covers the unused cols 8-31
            nc.gpsimd.memset(ones_sb[:, :], 1.0)
            nc.gpsimd.memset(vt_sb[:, :, C:C + 1], 1.0)
            nc.gpsimd.memset(ones_rows[:, :].bitcast(F32), 1.0)
            nc.gpsimd.memset(wqk_rep[:, :, :].bitcast(F32), 0.0)
            # pre-load the ScalarE activation table during the DMA wait
            nc.scalar.activation(warm_sb[0:1, :],
                                 ones_sb[0:1, 0:1].to_broadcast([1, 4]),
                                 mybir.ActivationFunctionType.Exp)
            # HAM warmup: PE_HAM un-throttles only after a long stretch of
            # busy activity windows; burn the otherwise-idle boot DMA wait on
            # dummy matmuls so the busy counter starts at ~1 us, not ~15 us
            if N_WARM:
                warm_ps = pp.tile([IB, GRP * JT], F32, tag="qkA",
                                  name="warmps")
                for i in range(N_WARM):
                    nc.tensor.matmul(warm_ps[0:C, 0:JT],
                                     lhsT=ones_rows[0:1, :],
                                     rhs=warm_rhs[0:1, :],
                                     start=(i == 0), stop=(i == N_WARM - 1))
            # on-chip transposes: Wq/Wk rows live in wqwk_raw[0:8] (rest
            # zeroed); DVE transposes 32x32 blocks
            for m in range(2):          # 0 = q, 1 = k
                for j in range(2):
                    nc.vector.transpose(
                        wqkT[32 * j:32 * j + 32, 32 * m:32 * m + 32],
                        wqwk_raw[0:32, m * C + 32 * j:m * C + 32 * j + 32])
            # free-dim-broadcast copies place each weight block at the three
            # 32-aligned column positions in one DVE instruction
            nc.vector.tensor_copy(
                wqk_rep[0:C, 0:GRP, 0:D],
                wqkT[0:C, None, 0:D].to_broadcast((C, GRP, D)))
            nc.vector.tensor_copy(
                wqk_rep[C:C + 1, 0:GRP, 0:D],
                bias_raw[0:1, None, 0:D].to_broadcast((1, GRP, D)))
            nc.vector.tensor_copy(wqk_rep[0:C, 3, 0:D],
                                  wqkT[0:C, 32:32 + D])
            nc.vector.tensor_copy(wqk_rep[C:C + 1, 3, 0:D],
                                  bias_raw[0:1, D:2 * D])

            x_rounded = [False] * 8
            wv_init = [False]

            def ensure_x(ch):
                """Round x chunk ch (512 wide) to f32r lazily so the startup
                chain doesn't queue behind the whole x preprocessing."""
                cs = slice(ch * JT, (ch + 1) * JT)
                if x_rounded[ch]:
                    return
                x_rounded[ch] = True
                nc.vector.tensor_copy(x_sb[0:C, cs], x_raw[:, cs])
                nc.gpsimd.memset(x_sb[C:C + 1, cs].bitcast(F32), 1.0)

            def emit_proj(ct, proj_tile):
                """Project q AND k for HW-chunk ct (512 wide) in ONE
                matmul: the replicated stationary lands q at partition
                groups {0, 32, 64} and k once at {96-103}; a single
                [104,512] DVE copy evacuates everything (garbage rows ride
                into never-read q_sb rows).  k is then scattered to its
                {0, 32, 64} positions by SWDGE DMAs (chunk c is first read
                in j-tile c, so the multi-us SWDGE latency is free)."""
                ensure_x(ct)
                js = slice(ct * JT, (ct + 1) * JT)
                p = proj_tile()
                nc.tensor.matmul(p[0:96 + D, :],
                                 lhsT=wqk_rep[:, :, :].rearrange(
                                     "p g c -> p (g c)")[:, 0:96 + D],
                                 rhs=x_sb[:, js], start=True, stop=True)
                nc.vector.tensor_copy(q_sb[:, js], p[0:96 + D, :])

            def scatter_k(ct, on_dve=False):
                js = slice(ct * JT, (ct + 1) * JT)
                for r in (0, 32, 64):
                    if on_dve:
                        nc.vector.tensor_copy(k_sb[r:r + D, js],
                                              q_sb[96:96 + D, js])
                    else:
                        nc.gpsimd.dma_start(out=k_sb[r:r + D, js],
                                            in_=q_sb[96:96 + D, js])

            def emit_vt_proj(vb, proj_tile):
                """Project vT' i-blocks vb*VB .. vb*VB+VB-1."""
                if not wv_init[0]:
                    wv_init[0] = True
                    for i in range(2):
                        for j in range(2):
                            nc.vector.transpose(
                                wvT[32 * j:32 * j + 32, 32 * i:32 * i + 32],
                                wv_raw[32 * i:32 * i + 32, 32 * j:32 * j + 32])
                    nc.vector.tensor_copy(wv_sb[0:C, :], wvT[:, :])
                    nc.vector.tensor_copy(wv_sb[C:C + 1, :],
                                          bias_raw[0:1, 2 * D:])
                ensure_x((vb * VB * IB) // JT)
                ensure_x(((vb + 1) * VB * IB - 1) // JT)
                pv = proj_tile()
                for u in range(VB):
                    ib = vb * VB + u
                    isl = slice(ib * IB, (ib + 1) * IB)
                    nc.tensor.matmul(pv[0:IB, u * C:(u + 1) * C],
                                     lhsT=x_sb[:, isl], rhs=wv_sb[:, :],
                                     start=True, stop=True)
                nc.vector.tensor_copy(
                    vt_sb[:, vb * VB:(vb + 1) * VB, 0:C],
                    pv[:, 0:VB * C].rearrange("p (v c) -> p v c", v=VB))

            def _compute():
                n_grp = (NI + GRP - 1) // GRP
                qk_done = 0
                vt_done = 0
                gidx = [0]         # global group counter (qk ring parity)
                step = [0]
                pend_av = []       # FIFO of (av, att, g, nb, js)
                pend_ep = []       # (av, js)

                def qk_tile():
                    # fp32 logits: TRN2 matmul can only write fp32 PSUM
                    # (16-bit PSUM output is TRN3-only), which pins the exp
                    # ring at 2x3 banks and GRP at 3
                    t = pp.tile([IB, GRP * JT], F32,
                                tag=("qkA" if gidx[0] % 2 == 0 else "qkB"),
                                name="qk")
                    gidx[0] += 1
                    return t

                def av_tile(jt):
                    return pp.tile([IB, JT], F32,
                                   tag=("avA" if jt % 2 == 0 else "avB"),
                                   name="av")

                def proj_tile():
                    # j-tile 0 projection scratch shares avB (av(jt1) is the
                    # next user of that bank, long after the last projection)
                    return pp.tile([IB, JT], F32, tag="avB", name="proj")

                def proj_tileA():
                    # chunk 0's k projection rides the avA bank, which is
                    # free until av(jt0)'s first accumulation at step 2 --
                    # this keeps it off pq(0)'s WAR chain so the first QK
                    # group isn't serialized behind two evacuations
                    return pp.tile([IB, JT], F32, tag="avA", name="projA")

                def ensure_vt(hi_block):
                    nonlocal vt_done
                    while vt_done * VB < hi_block:
                        emit_vt_proj(vt_done, proj_tile)
                        vt_done += 1

                def flush_av():
                    pav, patt, pg, pnb, pjs = pend_av.pop(0)
                    # one-group vt lookahead so the DVE evacuation is queued
                    # well before the AV matmuls that read it
                    ensure_vt(min(NI, pg * GRP + pnb + GRP))
                    for bi in range(pnb):
                        ib = pg * GRP + bi
                        nc.tensor.matmul(
                            pav[0:C + 1, :],
                            lhsT=vt_sb[:, ib, :],
                            rhs=patt[:, bi * JT:(bi + 1) * JT],
                            start=(ib == 0), stop=(ib == NI - 1))
                    if pg * GRP + pnb == NI:
                        pend_ep.append((pav, pjs))

                def flush_ep(final=False):
                    while pend_ep:
                        pav, pjs = pend_ep.pop(0)
                        # reciprocal straight off the PSUM denominator row
                        # (no den evacuation copy); the next j-tile
                        # accumulates into the other av bank, so the PE never
                        # stalls on this epilogue
                        if final:
                            # tail path: the qk ring is idle now, so a ones-
                            # stationary matmul broadcasts the reciprocal into
                            # a ring bank in ~0.5 us (the DMA broadcast below
                            # costs ~6.5 us of descriptor latency, hidden
                            # mid-run but fully exposed at the tail).  The
                            # 3.3 us DVE reciprocal is the tail's long pole,
                            # so run the epilogue in two 256-column halves:
                            # half 1's bcast/mult/add/DMA overlap half 2's
                            # reciprocal.
                            recip_r = wp.tile([IB, JT], F32R, tag="recipr")
                            bcps = pp.tile([IB, JT], F32, tag="qkA",
                                           name="bcps")
                            HJ = JT // 2
                            for h in range(2):
                                hs = slice(h * HJ, (h + 1) * HJ)
                                with nc.allow_low_precision(
                                        reason="f32r round of softmax recip"):
                                    nc.vector.reciprocal(recip_r[64:65, hs],
                                                         pav[64:65, hs])
                                nc.tensor.matmul(bcps[0:C, hs],
                                                 lhsT=ones_rows[64:65, 0:C],
                                                 rhs=recip_r[64:65, hs],
                                                 start=True, stop=True,
                                                 tile_position=(64, 0))
                            for h in range(2):
                                hs = slice(h * HJ, (h + 1) * HJ)
                                hjs = slice(pjs.start + h * HJ,
                                            pjs.start + (h + 1) * HJ)
                                bc_sb = wp.tile([C, HJ], F32, tag="bch")
                                nc.vector.tensor_copy(bc_sb[:, :],
                                                      bcps[0:C, hs])
                                oh = wp.tile([C, HJ], F32, tag="oh")
                                nc.vector.tensor_tensor(
                                    oh[:, :], pav[0:C, hs], bc_sb[:, :],
                                    op=mybir.AluOpType.mult)
                                nc.vector.tensor_tensor(
                                    oh[:, :], oh[:, :], x_raw[:, hjs],
                                    op=mybir.AluOpType.add)
                                nc.sync.dma_start(out=out_d[:, hjs],
                                                  in_=oh[:, :])
                            continue
                        else:
                            recip = wp.tile([IB, JT], F32, tag="recip")
                            nc.vector.reciprocal(recip[64:65, :],
                                                 pav[64:65, :])
                            bc_sb = wp.tile([C, JT], F32, tag="bc")
                            # partition-broadcast DMA: zero-step FREE dim on
                            # the src (the same 2 KB row read 64 times); a
                            # zero-step PARTITION dim is rejected by the DMA
                            # lowering.  ~6.5 us of completion latency, fully
                            # hidden by the next j-tile's compute
                            nc.sync.dma_start(
                                out=bc_sb[:, :],
                                in_=recip[64:65, None, :]
                                .to_broadcast((1, C, JT)))
                            bc_src = bc_sb[:, :]
                        o = wp.tile([C, JT], F32, tag="o")
                        nc.vector.tensor_tensor(o[:, :], pav[0:C, :],
                                                bc_src,
                                                op=mybir.AluOpType.mult)
                        nc.vector.tensor_tensor(o[:, :], o[:, :], x_raw[:, pjs],
                                                op=mybir.AluOpType.add)
                        # SWDGE (gpsimd) issue: that queue is idle mid-run
                        # and nothing waits on the store until the final
                        # drain, so its multi-us latency is free -- while
                        # the sync HWDGE queue stays dedicated to the
                        # broadcast DMA (sharing made the broadcast complete
                        # ~10 us late, thinning av-bank margins)
                        nc.gpsimd.dma_start(out=out_d[:, pjs], in_=o[:, :])

                # prologue: the fused chunk-0 projection precedes av(jt0)'s
                # allocation in the proj ring; its k scatter rides DVE (the
                # first QK group needs it right away)
                emit_proj(0, proj_tile)
                scatter_k(0, on_dve=True)
                qk_done = 1
                k_done = 1

                for jt in range(NJ):
                    js = slice(jt * JT, (jt + 1) * JT)
                    av = av_tile(jt)
                    for g in range(n_grp):
                        nb = min(GRP, NI - g * GRP)
                        if jt == 0:
                            # round x chunks two groups ahead of the proj
                            # front: the x-round CAST carries a coarse WAR
                            # wait on PE progress (Tile tracks x_sb at tensor
                            # granularity), which stalled the next projection
                            # by ~500-800 ns when emitted just-in-time
                            for ch in range(min(8, g + 3)):
                                ensure_x(ch)
                            # just-in-time q projections, one chunk ahead of
                            # the QK front; k chunk c is only read from
                            # j-tile c on, so the pk projections trickle one
                            # per group (halving early proj-bank WAR stalls)
                            hi_i = (g * GRP + nb) * IB
                            need = min(8, max(1, -(-hi_i // JT)) + 1)
                            while qk_done < need:
                                emit_proj(qk_done, proj_tile)
                                qk_done += 1

                            if g == n_grp - 1:
                                while qk_done < 8:
                                    emit_proj(qk_done, proj_tile)
                                    qk_done += 1
                                while k_done < 8:
                                    scatter_k(k_done)
                                    k_done += 1
                                # guarantee every avB-bank projection tile is
                                # emitted before av(jt1)'s allocation (the
                                # lazy flush_av path already reaches 32 here)
                                ensure_vt(NI)
                        qk = qk_tile()
                        for bi in range(nb):
                            ib = g * GRP + bi
                            isl = slice(ib * IB, (ib + 1) * IB)
                            nc.tensor.matmul(
                                qk[:, bi * JT:(bi + 1) * JT],
                                lhsT=q_sb[32 * bi:32 * bi + D, isl],
                                rhs=k_sb[32 * bi:32 * bi + D, js],
                                start=True, stop=True,
                                tile_position=(32 * bi, 0))
                        att = wp.tile([IB, GRP * JT], BF16, tag="att")
                        nc.scalar.activation(
                            att[:, 0:nb * JT], qk[:, 0:nb * JT],
                            mybir.ActivationFunctionType.Exp)
                        if jt == 0 and k_done < min(8, qk_done) and g >= 1:
                            # k chunk k_done's staging rows exist once its
                            # fused projection ran; scatter via SWDGE
                            scatter_k(k_done)
                            k_done += 1
                        flush_ep()
                        pend_av.append((av, att, g, nb, js))
                        while len(pend_av) > AV_LAG:
                            flush_av()
                        step[0] += 1
                while pend_av:
                    flush_av()
                    flush_ep(final=True)
                flush_ep(final=True)

            if loop_n:
                hints = (mybir.EngineType.PE, mybir.EngineType.Activation,
                         mybir.EngineType.DVE, mybir.EngineType.SP,
                         mybir.EngineType.Pool)
                with tc.For_i(0, loop_n, 1, hint_engines=hints):
                    for _ in range(bodies):
                        x_rounded[:] = [False] * 8
                        _compute()
            else:
                _compute()

    _fix_drain_waits(nc)
    return nc


_NC_CACHE = {}


def _get_nc():
    if "nc" not in _NC_CACHE:
        _NC_CACHE["nc"] = build_nc()
    return _NC_CACHE["nc"]


def kernel(**inputs) -> np.ndarray:
    x = np.ascontiguousarray(np.asarray(inputs["x"], dtype=np.float32))
    assert x.shape == (B, C, H, W), x.shape
    weights = {
        name: np.ascontiguousarray(np.asarray(inputs[name], dtype=np.float32))
        for name in ("Wq", "bq", "Wk", "bk", "Wv", "bv")
    }
    in_maps = [{"x": x[b].reshape(C, HW), **weights} for b in range(B)]
    nc = _get_nc()
    res = run_bass_kernel_spmd(nc, in_maps, core_ids=list(range(B)))
    out = np.stack([np.asarray(res.results[b]["out"]).reshape(C, H, W)
                    for b in range(B)])
    return out.astype(np.float32)
